# revision 1
# baseline (speedup 1.0000x reference)
"""3-layer GAT on Trainium2, 8 NeuronCores.

Strategy (dst-sharded):
  - Nodes padded to NPAD (mult of 8*128); core c owns a contiguous range of
    NPC nodes.  All edges (incl. self-loops on every padded node) are routed
    to the core that owns their *destination*, sorted by dst, grouped into
    dst-blocks of 128 destination nodes, and padded to chunks of 128 edges.
  - Per layer: each core computes h = y @ W for its own node rows (PE,
    bf16/f32r), plus per-head attention logits s,d (DVE).  It writes a
    "combined row" per node to DRAM: [h (bf16), s|d (f32 tail)], then an
    AllGather replicates the combined rows of all nodes to every core.
  - Edge phase: per dst-block, one dma_gather pulls the combined rows of the
    edge sources (h[src], s[src]) and a second tiny dma_gather pulls the
    tails of the edge destinations (d[dst]).  Softmax numerator
    exp(leaky_relu(s+d) - c) is computed per edge (c = per-core scalar
    upper bound on logits; softmax is shift-invariant so a per-core
    constant is exact since all edges of a dst live on one core).  The
    weighted segment-sum over incoming edges is a PE matmul with a
    host-precomputed one-hot mask S[e, dst_local], with exp folded into the
    gathered rows; the denominator uses the same mask with rhs = exp.
    Division, skip connection (y @ lin_W + b, PE), and ELU happen in the
    per-block epilogue; layer output is transposed (PE) into feat-major yT
    for the next layer's matmuls.
  - Layer 3: concat=False -> mean over 6 heads, no ELU; per-core rows DMA'd
    out, host concatenates and drops padding.
"""

import functools
import numpy as np
from contextlib import ExitStack

import ml_dtypes
import concourse.bass as bass
import concourse.bacc as bacc
import concourse.tile as tile
import concourse.masks as masks
from concourse import mybir
from concourse import library_config
from concourse._compat import cdiv

dt = mybir.dt
Alu = mybir.AluOpType
Act = mybir.ActivationFunctionType

BF16 = np.dtype(ml_dtypes.bfloat16)
NCORES = 8
P = 128

# layer configs: (F_in, F_out=H*C, H, C)
LAYERS = [
    (128, 1024, 4, 256),
    (1024, 1024, 4, 256),
    (1024, 384, 6, 64),
]
NEG_SLOPE = 0.2
OUT_DIM = 64
TAIL = 128  # tail units (bf16) appended to h in each combined row (256 B)


# ---------------------------------------------------------------------------
# host-side graph preprocessing
# ---------------------------------------------------------------------------

def _prep_graph(edge_index, n_pad):
    """Sort edges (plus self-loops on all padded nodes) by dst; bucket into
    dst-blocks of 128; pad each block's edge list to a globally uniform
    multiple of 128 (KMAX chunks, SPMD uniformity across cores)."""
    src = np.asarray(edge_index[0], dtype=np.int64)
    dst = np.asarray(edge_index[1], dtype=np.int64)
    loops = np.arange(n_pad, dtype=np.int64)
    src = np.concatenate([src, loops])
    dst = np.concatenate([dst, loops])

    order = np.argsort(dst, kind="stable")
    src, dst = src[order], dst[order]

    nblocks = n_pad // P  # global dst blocks
    blk = dst // P
    counts = np.bincount(blk, minlength=nblocks)
    kmax = int(cdiv(int(counts.max()), P))
    neb = kmax * P  # edges per block (padded)

    src_pad = np.zeros((nblocks, neb), dtype=np.int64)
    dst_pad = np.zeros((nblocks, neb), dtype=np.int64)
    valid = np.zeros((nblocks, neb), dtype=bool)
    starts = np.concatenate([[0], np.cumsum(counts)])
    for b in range(nblocks):
        c = counts[b]
        s0 = starts[b]
        src_pad[b, :c] = src[s0:s0 + c]
        dst_pad[b, :c] = dst[s0:s0 + c]
        valid[b, :c] = True

    # one-hot masks, layout [nblocks, 128 (e within chunk), kmax, 128 (m)]
    dst_local = (dst_pad - (np.arange(nblocks) * P)[:, None]).astype(np.int64)
    dst_local[~valid] = -1
    # per-edge local dst index in e-partition-major layout [nblocks, 128, kmax]
    dl = dst_local.reshape(nblocks, kmax, P).transpose(0, 2, 1)
    dl = np.ascontiguousarray(dl.astype(np.int16))

    return dict(kmax=kmax, neb=neb, src_pad=src_pad, dst_pad=dst_pad, dl=dl)


def _wrap_idx(a):
    # [n] int -> [16, n//16] int16 (wrapped in 16 partitions; device replicates)
    n = a.shape[0]
    assert n % 16 == 0
    w = a.reshape(n // 16, 16).T.astype(np.int16)
    return np.ascontiguousarray(w)


# ---------------------------------------------------------------------------
# bass program builder
# ---------------------------------------------------------------------------

def build_program(n_pad, kmax, stage=99):
    # stage gates for HW bisection: 1=phase1(L1) 2=+c 3=+AG 4=+skip
    # 5=+edge(L1) 6=+L2 99=full
    npc = n_pad // NCORES      # nodes per core
    nb = npc // P              # dst blocks per core
    ne = nb * kmax * P         # padded edges per core
    kq = cdiv(kmax, 4)         # gather piece size (chunks)
    pieces = [(k0, min(kq, kmax - k0)) for k0 in range(0, kmax, kq)]

    nc = bacc.Bacc("TRN2", target_bir_lowering=False, debug=False)

    f32, bf16, i16 = dt.float32, dt.bfloat16, dt.int16
    f32r = dt.float32r

    # ---------------- DRAM I/O ----------------
    xT = nc.dram_tensor("xT", [P, npc], bf16, kind="ExternalInput")
    W = []
    for li, (fi, fo, H, C) in enumerate(LAYERS):
        wdt = bf16
        f_skip = OUT_DIM if li == 2 else fo
        W.append(dict(
            W=nc.dram_tensor(f"W{li}", [fi, fo], wdt, kind="ExternalInput"),
            linW=nc.dram_tensor(f"linW{li}", [fi, f_skip], wdt,
                                kind="ExternalInput"),
            brow=nc.dram_tensor(f"brow{li}", [1, f_skip], wdt,
                                kind="ExternalInput"),
            aS=nc.dram_tensor(f"aS{li}", [1, fo], f32, kind="ExternalInput"),
            aD=nc.dram_tensor(f"aD{li}", [1, fo], f32, kind="ExternalInput"),
        ))
    idx_src = nc.dram_tensor("idx_src", [16, ne // 16], i16, kind="ExternalInput")
    idx_dst = nc.dram_tensor("idx_dst", [16, ne // 16], i16, kind="ExternalInput")
    dstloc = nc.dram_tensor("dstloc", [P, nb * kmax], f32, kind="ExternalInput")
    out_dram = nc.dram_tensor("out", [npc, OUT_DIM], f32, kind="ExternalOutput")

    comb_own, comb_full = [], []
    for li, (fi, fo, H, C) in enumerate(LAYERS):
        cw = fo + TAIL
        comb_own.append(nc.dram_tensor(f"comb_own{li}", [npc, cw], bf16))
        comb_full.append(
            nc.dram_tensor(f"comb_full{li}", [n_pad, cw], bf16,
                           addr_space="Shared"))

    replica_groups = [list(range(NCORES))]

    with tile.TileContext(nc) as tc, ExitStack() as ctx:
        const_pool = ctx.enter_context(tc.tile_pool(name="const", bufs=1))
        wpool = ctx.enter_context(tc.tile_pool(name="w", bufs=1))
        hpool = ctx.enter_context(tc.tile_pool(name="h", bufs=2))
        sdpool = ctx.enter_context(tc.tile_pool(name="sd", bufs=3))
        gpool = ctx.enter_context(tc.tile_pool(name="g", bufs=4))
        tpool = ctx.enter_context(tc.tile_pool(name="t", bufs=4))
        mpool = ctx.enter_context(tc.tile_pool(name="m", bufs=4))
        epool = ctx.enter_context(tc.tile_pool(name="e", bufs=4))
        ypool = ctx.enter_context(tc.tile_pool(name="y", bufs=2))
        yTpool = ctx.enter_context(tc.tile_pool(name="yT", bufs=1))
        skippool = ctx.enter_context(tc.tile_pool(name="skip", bufs=1))
        smallpool = ctx.enter_context(tc.tile_pool(name="small", bufs=4))
        psum_mm = ctx.enter_context(tc.tile_pool(name="psmm", bufs=2, space="PSUM"))
        psum_agg = ctx.enter_context(tc.tile_pool(name="psagg", bufs=1, space="PSUM"))
        psum_den = ctx.enter_context(tc.tile_pool(name="psden", bufs=1, space="PSUM"))
        psum_sm = ctx.enter_context(tc.tile_pool(name="pssm", bufs=1, space="PSUM"))

        nc.gpsimd.load_library(library_config.mlp)

        # constants
        ident = const_pool.tile([P, P], f32)
        masks.make_identity(nc, ident[:])
        ones_f32 = const_pool.tile([1, P], f32)
        nc.vector.memset(ones_f32[:], 1.0)
        ones_bf = const_pool.tile([1, P], bf16)
        nc.vector.memset(ones_bf[:], 1.0)

        # index tiles (persistent); replicate [16, C] -> [128, C] on device
        idxs_t = const_pool.tile([P, ne // 16], i16, tag="idxs")
        idxd_t = const_pool.tile([P, ne // 16], i16, tag="idxd")
        for gi in range(8):
            nc.sync.dma_start(idxs_t[16 * gi:16 * (gi + 1), :], idx_src[:])
            nc.sync.dma_start(idxd_t[16 * gi:16 * (gi + 1), :], idx_dst[:])
        dstloc_t = const_pool.tile([P, nb * kmax], f32, tag="dstloc")
        nc.sync.dma_start(dstloc_t[:], dstloc[:])
        iot32 = const_pool.tile([P, P], dt.int32, tag="iot32")
        nc.gpsimd.iota(iot32[:], pattern=[[1, P]], base=0, channel_multiplier=0)
        iot = const_pool.tile([P, P], f32, tag="iot")
        nc.vector.tensor_copy(iot[:], iot32[:])

        # xT resident for layer 1 (lhsT, f32)
        xT_sb = const_pool.tile([P, npc], bf16, tag="xT")
        nc.sync.dma_start(xT_sb[:], xT[:])

        yT_prev = None  # bf16 [128, fi//128, npc] for layers 2/3

        nlayers = (0 if stage == 0 else 1) if stage <= 5 else (2 if stage == 6 else len(LAYERS))
        for li, (fi, fo, H, C) in enumerate(LAYERS[:nlayers]):
            wdt = bf16
            kin = fi // P
            cw = fo + TAIL
            tail0 = fo
            f_skip = OUT_DIM if li == 2 else fo
            last = li == 2
            ones_row = ones_f32 if wdt == f32 else ones_bf

            def mm(ap):
                return ap

            def lhs(k, b):
                if li == 0:
                    return xT_sb[:, b * P:(b + 1) * P]
                return yT_prev[:, k, b * P:(b + 1) * P]

            # -------- load weights for this layer --------
            W_sb = wpool.tile([P, kin, fo], wdt, tag="W")
            nc.sync.dma_start(W_sb[:], W[li]["W"].rearrange("(k p) f -> p k f", p=P))
            linW_sb = wpool.tile([P, kin, f_skip], wdt, tag="linW")
            nc.sync.dma_start(
                linW_sb[:], W[li]["linW"].rearrange("(k p) f -> p k f", p=P))
            brow_sb = wpool.tile([1, f_skip], wdt, tag="brow")
            nc.sync.dma_start(brow_sb[:], W[li]["brow"][:])
            aS_sb = wpool.tile([P, fo], f32, tag="aS")
            aD_sb = wpool.tile([P, fo], f32, tag="aD")
            arow = wpool.tile([1, 2 * fo], f32, tag="arow")
            nc.sync.dma_start(arow[:, 0:fo], W[li]["aS"][:])
            nc.sync.dma_start(arow[:, fo:2 * fo], W[li]["aD"][:])
            for dst_sb, off in ((aS_sb, 0), (aD_sb, fo)):
                for h0 in range(0, fo, 512):
                    hw_ = min(512, fo - h0)
                    pa = psum_mm.tile([P, 512], f32, tag="mm", name="pa")
                    nc.tensor.matmul(pa[:, 0:hw_], ones_f32[:],
                                     arow[:, off + h0:off + h0 + hw_],
                                     start=True, stop=True)
                    nc.vector.tensor_copy(dst_sb[:, h0:h0 + hw_], pa[:, 0:hw_])

            # running per-partition max of s and d
            smax = smallpool.tile([P, 2], f32, tag="smax")

            # -------- phase 1: h = y@W, s/d logits, comb rows --------
            segs = [(h0, min(512, fo - h0)) for h0 in range(0, fo, 512)]
            for b in range(nb):
                ph = [psum_mm.tile([P, 512], f32, tag="mm", name=f"ph{si}")
                      for si in range(len(segs))]
                for si, (h0, hw_) in enumerate(segs):
                    for k in range(kin):
                        nc.tensor.matmul(
                            ph[si][:, 0:hw_], mm(lhs(k, b)),
                            mm(W_sb[:, k, h0:h0 + hw_]),
                            start=(k == 0), stop=(k == kin - 1))
                # s/d logits: per head reduce( h * a )
                sd = sdpool.tile([P, 2 * H], f32, tag="sd")
                scr = sdpool.tile([P, C], f32, tag="sdscr")
                for hh in range(H):
                    si, off = divmod(hh * C, 512)
                    for j, aB in enumerate((aS_sb, aD_sb)):
                        nc.vector.tensor_mul(scr[:], ph[si][:, off:off + C],
                                             aB[:, hh * C:(hh + 1) * C])
                        nc.vector.tensor_reduce(
                            sd[:, j * H + hh:j * H + hh + 1], scr[:],
                            axis=mybir.AxisListType.X, op=Alu.add)
                # track running max over (s cols, d cols)
                red = smallpool.tile([P, 2], f32, tag="red")
                nc.vector.tensor_reduce(red[:, 0:1], sd[:, 0:H],
                                        axis=mybir.AxisListType.X, op=Alu.max)
                nc.vector.tensor_reduce(red[:, 1:2], sd[:, H:2 * H],
                                        axis=mybir.AxisListType.X, op=Alu.max)
                if b == 0:
                    nc.vector.tensor_copy(smax[:], red[:])
                else:
                    nc.vector.tensor_max(smax[:], smax[:], red[:])
                # h -> bf16, write comb rows
                hbf = hpool.tile([P, fo], bf16, tag="hbf")
                for si, (h0, hw_) in enumerate(segs):
                    nc.vector.tensor_copy(hbf[:, h0:h0 + hw_], ph[si][:, 0:hw_])
                nc.sync.dma_start(comb_own[li][b * P:(b + 1) * P, 0:fo], hbf[:])
                nc.sync.dma_start(
                    comb_own[li][b * P:(b + 1) * P, tail0:tail0 + 4 * H],
                    sd[:].bitcast(bf16))

            if stage < 2 and li == nlayers - 1:
                break
            # -------- phase 2: scalar logit bound c --------
            csum = smallpool.tile([P, 1], f32, tag="csum")
            nc.vector.tensor_add(csum[:], smax[:, 0:1], smax[:, 1:2])
            ct = psum_sm.tile([1, P], f32, tag="sm")
            nc.tensor.transpose(ct[:], csum[:], ident[:])
            c1 = smallpool.tile([1, 1], f32, tag="c1")
            nc.vector.tensor_reduce(c1[:], ct[:], axis=mybir.AxisListType.X,
                                    op=Alu.max)
            pc = psum_sm.tile([P, 1], f32, tag="sm")
            nc.tensor.matmul(pc[:], ones_f32[:], c1[:], start=True, stop=True)
            cP = smallpool.tile([P, 1], f32, tag="cP")
            nc.vector.tensor_copy(cP[:], pc[:])

            if stage < 3 and li == nlayers - 1:
                break
            # -------- phase 3: AllGather combined rows --------
            nc.gpsimd.collective_compute(
                "AllGather", Alu.bypass, replica_groups=replica_groups,
                ins=[comb_own[li][:]], outs=[comb_full[li][:]])

            if stage < 4 and li == nlayers - 1:
                break
            # -------- phase 4: skip GEMM (overlaps AllGather) --------
            skip_sb = skippool.tile([P, nb, f_skip], bf16, tag="skip")
            sksegs = [(h0, min(512, f_skip - h0)) for h0 in range(0, f_skip, 512)]
            for b in range(nb):
                ps = [psum_mm.tile([P, 512], f32, tag="mm", name=f"ps{si}")
                      for si in range(len(sksegs))]
                for si, (h0, hw_) in enumerate(sksegs):
                    for k in range(kin):
                        nc.tensor.matmul(
                            ps[si][:, 0:hw_], mm(lhs(k, b)),
                            mm(linW_sb[:, k, h0:h0 + hw_]),
                            start=(k == 0), stop=False)
                    nc.tensor.matmul(
                        ps[si][:, 0:hw_], mm(ones_row[:]),
                        mm(brow_sb[:, h0:h0 + hw_]),
                        start=False, stop=True)
                    nc.vector.tensor_copy(skip_sb[:, b, h0:h0 + hw_],
                                          ps[si][:, 0:hw_])

            if stage < 5 and li == nlayers - 1:
                break
            # -------- phase 5: edge phase per dst block --------
            if not last:
                yT_new = yTpool.tile([P, fo // P, npc], bf16,
                                     tag=f"yT{li % 2}", name=f"yT_new{li}")
            else:
                yT_new = None
            comb_ap = comb_full[li]
            for b in range(nb):
                pagg = [psum_agg.tile([P, 512], f32, tag="pagg",
                                      name=f"pagg{si}")
                        for si in range(len(segs))]
                pden = psum_den.tile([P, H], f32, tag="pden")
                for (k0, kh) in pieces:
                    e0 = (b * kmax + k0) * P
                    n_idx = kh * P
                    G = gpool.tile([P, kq, cw], bf16, tag="G")
                    nc.gpsimd.dma_gather(
                        G[:, 0:kh, :], comb_ap[:, :],
                        idxs_t[:, e0 // 16:(e0 + n_idx) // 16],
                        n_idx, n_idx, cw, single_packet=False)
                    T = tpool.tile([P, kq, TAIL], bf16, tag="T")
                    nc.gpsimd.dma_gather(
                        T[:, 0:kh, :], comb_ap[:, tail0:tail0 + TAIL],
                        idxd_t[:, e0 // 16:(e0 + n_idx) // 16],
                        n_idx, n_idx, TAIL, elem_step=cw,
                        single_packet=False)
                    Sm = mpool.tile([P, kq, P], bf16, tag="Sm")
                    for k in range(kh):
                        gk = b * kmax + k0 + k
                        nc.vector.tensor_scalar(
                            Sm[:, k, :], iot[:], dstloc_t[:, gk:gk + 1], None,
                            op0=Alu.is_equal)

                    # logits -> exp
                    sv = G[:, 0:kh, fo:fo + 2 * H].bitcast(f32)   # [P, kh, H]
                    dv = T[:, 0:kh, 2 * H:4 * H].bitcast(f32)     # [P, kh, H]
                    ee = epool.tile([P, kq, H], f32, tag="ee")
                    nc.vector.tensor_add(ee[:, 0:kh, :], sv, dv)
                    nc.vector.scalar_tensor_tensor(
                        ee[:, 0:kh, :], ee[:, 0:kh, :], NEG_SLOPE,
                        ee[:, 0:kh, :], op0=Alu.mult, op1=Alu.max)
                    nc.vector.tensor_scalar(
                        ee[:, 0:kh, :], ee[:, 0:kh, :], cP[:, 0:1], 60.0,
                        op0=Alu.subtract, op1=Alu.min)
                    ex = epool.tile([P, kq, H], bf16, tag="ex")
                    nc.scalar.activation(ex[:, 0:kh, :], ee[:, 0:kh, :], Act.Exp)
                    # scale gathered h rows by exp (per head), in place
                    gh = G[:, 0:kh, 0:fo].rearrange("p k (h c) -> p k h c", h=H)
                    nc.vector.tensor_mul(
                        gh, gh,
                        ex[:, 0:kh, :].unsqueeze(3).broadcast_to([P, kh, H, C]))
                    # segment-sum via mask matmuls
                    for k in range(kh):
                        kk = k0 + k
                        st, sp = kk == 0, kk == kmax - 1
                        for si, (h0, hw_) in enumerate(segs):
                            nc.tensor.matmul(
                                pagg[si][:, 0:hw_], Sm[:, k, :],
                                G[:, k, h0:h0 + hw_],
                                start=st, stop=sp)
                        nc.tensor.matmul(pden[:], Sm[:, k, :], ex[:, k, :],
                                         start=st, stop=sp)

                # epilogue for block b
                rden = smallpool.tile([P, H], f32, tag="rden")
                nc.vector.reciprocal(rden[:], pden[:])
                yf = ypool.tile([P, fo], f32, tag="yf")
                for si, (h0, hw_) in enumerate(segs):
                    nh = hw_ // C
                    hh0 = h0 // C
                    nc.vector.tensor_mul(
                        yf[:, h0:h0 + hw_].rearrange("p (h c) -> p h c", h=nh),
                        pagg[si][:, 0:hw_].rearrange("p (h c) -> p h c", h=nh),
                        rden[:, hh0:hh0 + nh].unsqueeze(2)
                            .broadcast_to([P, nh, C]))
                if not last:
                    nc.vector.tensor_add(yf[:], yf[:], skip_sb[:, b, :])
                    # ELU: y = max(yf,0) + exp(min(yf,0)) - 1
                    mn = ypool.tile([P, fo], f32, tag="mn")
                    nc.vector.tensor_scalar_min(mn[:], yf[:], 0.0)
                    nc.scalar.activation(mn[:], mn[:], Act.Exp)
                    nc.vector.scalar_tensor_tensor(
                        yf[:], yf[:], 0.0, mn[:], op0=Alu.max, op1=Alu.add)
                    nc.vector.tensor_scalar_add(yf[:], yf[:], -1.0)
                    # transpose into yT_new
                    for j in range(fo // P):
                        pt = psum_sm.tile([P, P], f32, tag="sm")
                        nc.tensor.transpose(pt[:], yf[:, j * P:(j + 1) * P],
                                            ident[:])
                        nc.vector.tensor_copy(yT_new[:, j, b * P:(b + 1) * P],
                                              pt[:])
                else:
                    # mean over heads + skip
                    yo = ypool.tile([P, OUT_DIM], f32, tag="yo")
                    nc.vector.tensor_reduce(
                        yo[:], yf[:].rearrange("p (h c) -> p c h", h=H),
                        axis=mybir.AxisListType.X, op=Alu.add)
                    nc.vector.tensor_scalar_mul(yo[:], yo[:], 1.0 / H)
                    nc.vector.tensor_add(yo[:], yo[:], skip_sb[:, b, :])
                    nc.sync.dma_start(out_dram[b * P:(b + 1) * P, :], yo[:])

            yT_prev = yT_new

        if stage < 99 and stage != 5 or (stage <= 6 and stage >= 5):
            dummy = const_pool.tile([P, OUT_DIM], f32, tag="dummy")
            nc.vector.memset(dummy[:], 1.0)
            for b in range(npc // P):
                nc.sync.dma_start(out_dram[b * P:(b + 1) * P, :], dummy[:])

    nc.compile()
    return nc


# ---------------------------------------------------------------------------
# host wrapper
# ---------------------------------------------------------------------------

@functools.lru_cache(maxsize=2)
def _cached_program(n_pad, kmax):
    return build_program(n_pad, kmax)


def _replicate_row(v):
    v = np.asarray(v, np.float32).reshape(-1)
    return np.ascontiguousarray(np.broadcast_to(v[None, :], (P, v.shape[0])))


def make_in_maps(x, edge_index, weights):
    """weights: list of 3 dicts with keys W, linW, brow, aS, aD (numpy f32)."""
    n = x.shape[0]
    n_pad = cdiv(n, NCORES * P) * NCORES * P
    npc = n_pad // NCORES
    nb = npc // P

    g = _prep_graph(edge_index, n_pad)

    x_pad = np.zeros((n_pad, x.shape[1]), np.float32)
    x_pad[:n] = np.asarray(x, np.float32)
    xT_all = np.ascontiguousarray(x_pad.T)

    layer_w = []
    for li, lw in enumerate(weights):
        wdt = BF16
        layer_w.append(dict(
            W=np.ascontiguousarray(np.asarray(lw["W"], np.float32).astype(wdt)),
            linW=np.ascontiguousarray(
                np.asarray(lw["linW"], np.float32).astype(wdt)),
            brow=np.ascontiguousarray(
                np.asarray(lw["brow"], np.float32).astype(wdt)[None, :]),
            aS=np.asarray(lw["aS"], np.float32).reshape(1, -1),
            aD=np.asarray(lw["aD"], np.float32).reshape(1, -1),
        ))

    in_maps = []
    for c in range(NCORES):
        blo, bhi = c * nb, (c + 1) * nb
        nbc = bhi - blo
        kmax = g["kmax"]
        m = dict(
            xT=np.ascontiguousarray(
                xT_all[:, c * npc:(c + 1) * npc].astype(BF16)),
            idx_src=_wrap_idx(g["src_pad"][blo:bhi].reshape(-1)),
            idx_dst=_wrap_idx(g["dst_pad"][blo:bhi].reshape(-1)),
            dstloc=np.ascontiguousarray(
                g["dl"][blo:bhi].transpose(1, 0, 2).reshape(P, nbc * kmax)
                .astype(np.float32)),
        )
        for li, lw in enumerate(layer_w):
            for key in ("W", "linW", "brow", "aS", "aD"):
                m[f"{key}{li}" if key != "W" else f"W{li}"] = lw[key]
        in_maps.append(m)
    return in_maps, g, n_pad


def _weights_from_kwargs(W1, a1_src, a1_dst, b1, lin1_W, lin1_b,
                         W2, a2_src, a2_dst, b2, lin2_W, lin2_b,
                         W3, a3_src, a3_dst, b3, lin3_W, lin3_b):
    return [
        dict(W=W1, linW=lin1_W, brow=np.asarray(b1) + np.asarray(lin1_b),
             aS=a1_src, aD=a1_dst),
        dict(W=W2, linW=lin2_W, brow=np.asarray(b2) + np.asarray(lin2_b),
             aS=a2_src, aD=a2_dst),
        dict(W=W3, linW=lin3_W, brow=np.asarray(b3) + np.asarray(lin3_b),
             aS=a3_src, aD=a3_dst),
    ]


def run_gat(inputs, trace=False, **run_kwargs):
    from concourse.bass_utils import run_bass_kernel_spmd

    kw = {k: inputs[k] for k in (
        "W1", "a1_src", "a1_dst", "b1", "lin1_W", "lin1_b",
        "W2", "a2_src", "a2_dst", "b2", "lin2_W", "lin2_b",
        "W3", "a3_src", "a3_dst", "b3", "lin3_W", "lin3_b")}
    weights = _weights_from_kwargs(**kw)
    x, edge_index = inputs["x"], inputs["edge_index"]
    in_maps, g, n_pad = make_in_maps(x, edge_index, weights)
    nc = _cached_program(n_pad, g["kmax"])
    res = run_bass_kernel_spmd(nc, in_maps, list(range(NCORES)),
                               trace=trace, **run_kwargs)
    out = np.concatenate([res.results[c]["out"] for c in range(NCORES)],
                         axis=0)
    n = x.shape[0]
    return np.ascontiguousarray(out[:n]).astype(np.float32), res


def kernel(**inputs):
    return run_gat(inputs)[0]



# revision 12
# speedup vs baseline: 1.1432x; 1.1432x over previous
"""3-layer GAT on Trainium2, 8 NeuronCores.

Strategy (dst-sharded):
  - Nodes padded to NPAD (mult of 8*128); core c owns a contiguous range of
    NPC nodes.  All edges (incl. self-loops on every padded node) are routed
    to the core that owns their *destination*, sorted by dst, grouped into
    dst-blocks of 128 destination nodes, and padded to chunks of 128 edges.
  - Per layer: each core computes h = y @ W for its own node rows (PE,
    bf16/f32r), plus per-head attention logits s,d (DVE).  It writes a
    "combined row" per node to DRAM: [h (bf16), s|d (f32 tail)], then an
    AllGather replicates the combined rows of all nodes to every core.
  - Edge phase: per dst-block, one dma_gather pulls the combined rows of the
    edge sources (h[src], s[src]) and a second tiny dma_gather pulls the
    tails of the edge destinations (d[dst]).  Softmax numerator
    exp(leaky_relu(s+d) - c) is computed per edge (c = per-core scalar
    upper bound on logits; softmax is shift-invariant so a per-core
    constant is exact since all edges of a dst live on one core).  The
    weighted segment-sum over incoming edges is a PE matmul with a
    host-precomputed one-hot mask S[e, dst_local], with exp folded into the
    gathered rows; the denominator uses the same mask with rhs = exp.
    Division, skip connection (y @ lin_W + b, PE), and ELU happen in the
    per-block epilogue; layer output is transposed (PE) into feat-major yT
    for the next layer's matmuls.
  - Layer 3: concat=False -> mean over 6 heads, no ELU; per-core rows DMA'd
    out, host concatenates and drops padding.
"""

import functools
import numpy as np
from contextlib import ExitStack

import ml_dtypes
import concourse.bass as bass
import concourse.bacc as bacc
import concourse.tile as tile
import concourse.masks as masks
from concourse import mybir
from concourse import library_config
from concourse._compat import cdiv

dt = mybir.dt
Alu = mybir.AluOpType
Act = mybir.ActivationFunctionType

BF16 = np.dtype(ml_dtypes.bfloat16)
NCORES = 8
P = 128

# layer configs: (F_in, F_out=H*C, H, C)
LAYERS = [
    (128, 1024, 4, 256),
    (1024, 1024, 4, 256),
    (1024, 384, 6, 64),
]
NEG_SLOPE = 0.2
OUT_DIM = 64


def _tail(H):
    # tail units (bf16) appended to h in each combined row; holds s|d logits
    # as bitcast f32. dma_gather requires gathered rows to be 256B multiples,
    # so the tail is 128 units (256 B).
    return 128


def _ilperm(H, C):
    # column permutation: interleaved col j = c*H + h  <-  head-major h*C + c
    j = np.arange(H * C)
    return (j % H) * C + j // H


# ---------------------------------------------------------------------------
# host-side graph preprocessing
# ---------------------------------------------------------------------------

def _prep_graph(edge_index, n_pad):
    """Sort edges (plus self-loops on all padded nodes) by dst; bucket into
    dst-blocks of 128; pad each block's edge list to a globally uniform
    multiple of 128 (KMAX chunks, SPMD uniformity across cores)."""
    src = np.asarray(edge_index[0], dtype=np.int64)
    dst = np.asarray(edge_index[1], dtype=np.int64)
    loops = np.arange(n_pad, dtype=np.int64)
    src = np.concatenate([src, loops])
    dst = np.concatenate([dst, loops])

    order = np.argsort(dst, kind="stable")
    src, dst = src[order], dst[order]

    nblocks = n_pad // P  # global dst blocks
    blk = dst // P
    counts = np.bincount(blk, minlength=nblocks)
    kmax = int(cdiv(int(counts.max()), P))
    neb = kmax * P  # edges per block (padded)

    src_pad = np.zeros((nblocks, neb), dtype=np.int64)
    dst_pad = np.zeros((nblocks, neb), dtype=np.int64)
    valid = np.zeros((nblocks, neb), dtype=bool)
    starts = np.concatenate([[0], np.cumsum(counts)])
    for b in range(nblocks):
        c = counts[b]
        s0 = starts[b]
        src_pad[b, :c] = src[s0:s0 + c]
        dst_pad[b, :c] = dst[s0:s0 + c]
        valid[b, :c] = True

    # one-hot masks, layout [nblocks, 128 (e within chunk), kmax, 128 (m)]
    dst_local = (dst_pad - (np.arange(nblocks) * P)[:, None]).astype(np.int64)
    dst_local[~valid] = -1
    # per-edge local dst index in e-partition-major layout [nblocks, 128, kmax]
    dl = dst_local.reshape(nblocks, kmax, P).transpose(0, 2, 1)
    dl = np.ascontiguousarray(dl.astype(np.int16))

    return dict(kmax=kmax, neb=neb, src_pad=src_pad, dst_pad=dst_pad, dl=dl)


def _wrap_idx(a):
    # [n] int -> [16, n//16] int16 (wrapped in 16 partitions; device replicates)
    n = a.shape[0]
    assert n % 16 == 0
    w = a.reshape(n // 16, 16).T.astype(np.int16)
    return np.ascontiguousarray(w)


# ---------------------------------------------------------------------------
# bass program builder
# ---------------------------------------------------------------------------

def build_program(n_pad, kmax, stage=99):
    # stage gates for HW bisection: 1=phase1(L1) 2=+c 3=+AG 4=+skip
    # 5=+edge(L1) 6=+L2 99=full
    npc = n_pad // NCORES      # nodes per core
    nb = npc // P              # dst blocks per core
    ne = nb * kmax * P         # padded edges per core
    kq = cdiv(kmax, 4)         # gather piece size (chunks)
    pieces = [(k0, min(kq, kmax - k0)) for k0 in range(0, kmax, kq)]

    nc = bacc.Bacc("TRN2", target_bir_lowering=False, debug=False)

    f32, bf16, i16 = dt.float32, dt.bfloat16, dt.int16
    f32r = dt.float32r

    # ---------------- DRAM I/O ----------------
    xT = nc.dram_tensor("xT", [P, npc], bf16, kind="ExternalInput")
    W = []
    for li, (fi, fo, H, C) in enumerate(LAYERS):
        wdt = bf16
        f_skip = OUT_DIM if li == 2 else fo
        W.append(dict(
            W=nc.dram_tensor(f"W{li}", [fi, fo], wdt, kind="ExternalInput"),
            linW=nc.dram_tensor(f"linW{li}", [fi, f_skip], wdt,
                                kind="ExternalInput"),
            brow=nc.dram_tensor(f"brow{li}", [1, f_skip], wdt,
                                kind="ExternalInput"),
            aS=nc.dram_tensor(f"aS{li}", [1, fo], f32, kind="ExternalInput"),
            aD=nc.dram_tensor(f"aD{li}", [1, fo], f32, kind="ExternalInput"),
        ))
    idx_src = nc.dram_tensor("idx_src", [16, ne // 16], i16, kind="ExternalInput")
    idx_dst = nc.dram_tensor("idx_dst", [16, ne // 16], i16, kind="ExternalInput")
    dstloc = nc.dram_tensor("dstloc", [P, nb * kmax], f32, kind="ExternalInput")
    out_dram = nc.dram_tensor("out", [npc, OUT_DIM], f32, kind="ExternalOutput")

    comb_own, comb_full = [], []
    for li, (fi, fo, H, C) in enumerate(LAYERS):
        cw = fo + _tail(H)
        comb_own.append(nc.dram_tensor(f"comb_own{li}", [npc, cw], bf16))
        comb_full.append(
            nc.dram_tensor(f"comb_full{li}", [n_pad, cw], bf16,
                           addr_space="Shared"))

    replica_groups = [list(range(NCORES))]

    with tile.TileContext(nc) as tc, ExitStack() as ctx:
        const_pool = ctx.enter_context(tc.tile_pool(name="const", bufs=1))
        wpool = ctx.enter_context(tc.tile_pool(name="w", bufs=1))
        hpool = ctx.enter_context(tc.tile_pool(name="h", bufs=2))
        sdpool = ctx.enter_context(tc.tile_pool(name="sd", bufs=3))
        scrpool = ctx.enter_context(tc.tile_pool(name="scr", bufs=1))
        gpool = ctx.enter_context(tc.tile_pool(name="g", bufs=4))
        tpool = ctx.enter_context(tc.tile_pool(name="t", bufs=4))
        mpool = ctx.enter_context(tc.tile_pool(name="m", bufs=4))
        epool = ctx.enter_context(tc.tile_pool(name="e", bufs=4))
        ypool = ctx.enter_context(tc.tile_pool(name="y", bufs=2))
        yTpool = ctx.enter_context(tc.tile_pool(name="yT", bufs=1))
        skippool = ctx.enter_context(tc.tile_pool(name="skip", bufs=1))
        smallpool = ctx.enter_context(tc.tile_pool(name="small", bufs=4))
        psum_mm = ctx.enter_context(tc.tile_pool(name="psmm", bufs=2, space="PSUM"))
        psum_agg = ctx.enter_context(tc.tile_pool(name="psagg", bufs=1, space="PSUM"))
        psum_den = ctx.enter_context(tc.tile_pool(name="psden", bufs=1, space="PSUM"))
        psum_sm = ctx.enter_context(tc.tile_pool(name="pssm", bufs=1, space="PSUM"))

        nc.gpsimd.load_library(library_config.mlp)

        # constants
        ident = const_pool.tile([P, P], f32)
        masks.make_identity(nc, ident[:])
        ones_f32 = const_pool.tile([1, P], f32)
        nc.vector.memset(ones_f32[:], 1.0)
        ones_bf = const_pool.tile([1, P], bf16)
        nc.vector.memset(ones_bf[:], 1.0)

        # index tiles (persistent); replicate [16, C] -> [128, C] on device
        idxs_t = const_pool.tile([P, ne // 16], i16, tag="idxs")
        idxd_t = const_pool.tile([P, ne // 16], i16, tag="idxd")
        for gi in range(8):
            nc.sync.dma_start(idxs_t[16 * gi:16 * (gi + 1), :], idx_src[:])
            nc.sync.dma_start(idxd_t[16 * gi:16 * (gi + 1), :], idx_dst[:])
        dstloc_t = const_pool.tile([P, nb * kmax], f32, tag="dstloc")
        nc.sync.dma_start(dstloc_t[:], dstloc[:])
        iot32 = const_pool.tile([P, P], dt.int32, tag="iot32")
        nc.gpsimd.iota(iot32[:], pattern=[[1, P]], base=0, channel_multiplier=0)
        iot = const_pool.tile([P, P], f32, tag="iot")
        nc.vector.tensor_copy(iot[:], iot32[:])

        # xT resident for layer 1 (lhsT, f32)
        xT_sb = const_pool.tile([P, npc], bf16, tag="xT")
        nc.sync.dma_start(xT_sb[:], xT[:])

        yT_prev = None  # bf16 [128, fi//128, npc] for layers 2/3

        nlayers = (0 if stage == 0 else 1) if stage <= 5 else (2 if stage == 6 else len(LAYERS))
        for li, (fi, fo, H, C) in enumerate(LAYERS[:nlayers]):
            wdt = bf16
            kin = fi // P
            TAIL = _tail(H)
            cw = fo + TAIL
            tail0 = fo
            f_skip = OUT_DIM if li == 2 else fo
            last = li == 2
            ones_row = ones_f32 if wdt == f32 else ones_bf

            def mm(ap):
                return ap

            def lhs(k, b):
                if li == 0:
                    return xT_sb[:, b * P:(b + 1) * P]
                return yT_prev[:, k, b * P:(b + 1) * P]

            # -------- load weights for this layer --------
            W_sb = wpool.tile([P, kin, fo], wdt, tag="W")
            nc.sync.dma_start(W_sb[:], W[li]["W"].rearrange("(k p) f -> p k f", p=P))
            linW_sb = wpool.tile([P, kin, f_skip], wdt, tag="linW")
            nc.sync.dma_start(
                linW_sb[:], W[li]["linW"].rearrange("(k p) f -> p k f", p=P))
            brow_sb = wpool.tile([1, f_skip], wdt, tag="brow")
            nc.sync.dma_start(brow_sb[:], W[li]["brow"][:])
            aS_sb = wpool.tile([P, fo], f32, tag="aS")
            aD_sb = wpool.tile([P, fo], f32, tag="aD")
            arow = wpool.tile([1, 2 * fo], f32, tag="arow")
            nc.sync.dma_start(arow[:, 0:fo], W[li]["aS"][:])
            nc.sync.dma_start(arow[:, fo:2 * fo], W[li]["aD"][:])
            for dst_sb, off in ((aS_sb, 0), (aD_sb, fo)):
                for h0 in range(0, fo, 512):
                    hw_ = min(512, fo - h0)
                    pa = psum_mm.tile([P, 512], f32, tag="mm", name="pa")
                    nc.tensor.matmul(pa[:, 0:hw_], ones_f32[:],
                                     arow[:, off + h0:off + h0 + hw_],
                                     start=True, stop=True)
                    nc.vector.tensor_copy(dst_sb[:, h0:h0 + hw_], pa[:, 0:hw_])

            # running per-partition max of s and d
            smax = smallpool.tile([P, 2], f32, tag="smax")

            # -------- phase 1: h = y@W, s/d logits, comb rows --------
            segs = [(h0, min(512, fo - h0)) for h0 in range(0, fo, 512)]
            for b in range(nb):
                ph = [psum_mm.tile([P, 512], f32, tag="mm", name=f"ph{si}")
                      for si in range(len(segs))]
                for si, (h0, hw_) in enumerate(segs):
                    for k in range(kin):
                        nc.tensor.matmul(
                            ph[si][:, 0:hw_], mm(lhs(k, b)),
                            mm(W_sb[:, k, h0:h0 + hw_]),
                            start=(k == 0), stop=(k == kin - 1))
                # s/d logits: h * a, then per-head reduce over the strided
                # (c-major interleaved) view
                sd = sdpool.tile([P, 2 * H], f32, tag="sd")
                scr = scrpool.tile([P, 2, fo], f32, tag="sdscr")
                for j, aB in enumerate((aS_sb, aD_sb)):
                    for si, (h0, hw_) in enumerate(segs):
                        nc.vector.tensor_mul(scr[:, j, h0:h0 + hw_],
                                             ph[si][:, 0:hw_],
                                             aB[:, h0:h0 + hw_])
                    nc.vector.tensor_reduce(
                        sd[:, j * H:(j + 1) * H].unsqueeze(2),
                        scr[:, j, :].rearrange("p (c h) -> p h c", h=H),
                        axis=mybir.AxisListType.X, op=Alu.add)
                # track running max over (s cols, d cols)
                red = smallpool.tile([P, 2], f32, tag="red")
                nc.vector.tensor_reduce(red[:, 0:1], sd[:, 0:H],
                                        axis=mybir.AxisListType.X, op=Alu.max)
                nc.vector.tensor_reduce(red[:, 1:2], sd[:, H:2 * H],
                                        axis=mybir.AxisListType.X, op=Alu.max)
                if b == 0:
                    nc.vector.tensor_copy(smax[:], red[:])
                else:
                    nc.vector.tensor_max(smax[:], smax[:], red[:])
                # h -> bf16, write comb rows
                hbf = hpool.tile([P, fo], bf16, tag="hbf")
                for si, (h0, hw_) in enumerate(segs):
                    nc.vector.tensor_copy(hbf[:, h0:h0 + hw_], ph[si][:, 0:hw_])
                nc.sync.dma_start(comb_own[li][b * P:(b + 1) * P, 0:fo], hbf[:])
                nc.sync.dma_start(
                    comb_own[li][b * P:(b + 1) * P, tail0:tail0 + 4 * H],
                    sd[:].bitcast(bf16))

            if stage < 2 and li == nlayers - 1:
                break
            # -------- phase 2: scalar logit bound c --------
            csum = smallpool.tile([P, 1], f32, tag="csum")
            nc.vector.tensor_add(csum[:], smax[:, 0:1], smax[:, 1:2])
            ct = psum_sm.tile([1, P], f32, tag="sm")
            nc.tensor.transpose(ct[:], csum[:], ident[:])
            c1 = smallpool.tile([1, 1], f32, tag="c1")
            nc.vector.tensor_reduce(c1[:], ct[:], axis=mybir.AxisListType.X,
                                    op=Alu.max)
            pc = psum_sm.tile([P, 1], f32, tag="sm")
            nc.tensor.matmul(pc[:], ones_f32[:], c1[:], start=True, stop=True)
            cP = smallpool.tile([P, 1], f32, tag="cP")
            nc.vector.tensor_copy(cP[:], pc[:])

            if stage < 3 and li == nlayers - 1:
                break
            # -------- phase 3: AllGather combined rows --------
            nc.gpsimd.collective_compute(
                "AllGather", Alu.bypass, replica_groups=replica_groups,
                ins=[comb_own[li][:]], outs=[comb_full[li][:]])

            if stage < 4 and li == nlayers - 1:
                break
            # -------- phase 4: skip GEMM (overlaps AllGather) --------
            skip_sb = skippool.tile([P, nb, f_skip], bf16, tag="skip")
            sksegs = [(h0, min(512, f_skip - h0)) for h0 in range(0, f_skip, 512)]
            for b in range(nb):
                ps = [psum_mm.tile([P, 512], f32, tag="mm", name=f"ps{si}")
                      for si in range(len(sksegs))]
                for si, (h0, hw_) in enumerate(sksegs):
                    for k in range(kin):
                        nc.tensor.matmul(
                            ps[si][:, 0:hw_], mm(lhs(k, b)),
                            mm(linW_sb[:, k, h0:h0 + hw_]),
                            start=(k == 0), stop=False)
                    nc.tensor.matmul(
                        ps[si][:, 0:hw_], mm(ones_row[:]),
                        mm(brow_sb[:, h0:h0 + hw_]),
                        start=False, stop=True)
                    nc.vector.tensor_copy(skip_sb[:, b, h0:h0 + hw_],
                                          ps[si][:, 0:hw_])

            if stage < 5 and li == nlayers - 1:
                break
            # -------- phase 5: edge phase per dst block --------
            if not last:
                yT_new = yTpool.tile([P, fo // P, npc], bf16,
                                     tag=f"yT{li % 2}", name=f"yT_new{li}")
            else:
                yT_new = None
            comb_ap = comb_full[li]
            for b in range(nb):
                pagg = [psum_agg.tile([P, 512], f32, tag="pagg",
                                      name=f"pagg{si}")
                        for si in range(len(segs))]
                pden = psum_den.tile([P, H], f32, tag="pden")
                for (k0, kh) in pieces:
                    e0 = (b * kmax + k0) * P
                    n_idx = kh * P
                    G = gpool.tile([P, kq, cw], bf16, tag="G")
                    nc.gpsimd.dma_gather(
                        G[:, 0:kh, :], comb_ap[:, :],
                        idxs_t[:, e0 // 16:(e0 + n_idx) // 16],
                        n_idx, n_idx, cw, single_packet=False)
                    T = tpool.tile([P, kq, TAIL], bf16, tag="T")
                    nc.gpsimd.dma_gather(
                        T[:, 0:kh, :], comb_ap[:, tail0:tail0 + TAIL],
                        idxd_t[:, e0 // 16:(e0 + n_idx) // 16],
                        n_idx, n_idx, TAIL, elem_step=cw,
                        single_packet=False)
                    Sm = mpool.tile([P, kq, P], bf16, tag="Sm")
                    for k in range(kh):
                        gk = b * kmax + k0 + k
                        nc.vector.tensor_scalar(
                            Sm[:, k, :], iot[:], dstloc_t[:, gk:gk + 1], None,
                            op0=Alu.is_equal)

                    # logits -> exp
                    sv = G[:, 0:kh, fo:fo + 2 * H].bitcast(f32)   # [P, kh, H]
                    dv = T[:, 0:kh, 2 * H:4 * H].bitcast(f32)     # [P, kh, H]
                    ee = epool.tile([P, kq, H], f32, tag="ee")
                    nc.vector.tensor_add(ee[:, 0:kh, :], sv, dv)
                    nc.vector.scalar_tensor_tensor(
                        ee[:, 0:kh, :], ee[:, 0:kh, :], NEG_SLOPE,
                        ee[:, 0:kh, :], op0=Alu.mult, op1=Alu.max)
                    nc.vector.tensor_scalar(
                        ee[:, 0:kh, :], ee[:, 0:kh, :], cP[:, 0:1], 60.0,
                        op0=Alu.subtract, op1=Alu.min)
                    ex = epool.tile([P, kq, H], bf16, tag="ex")
                    nc.scalar.activation(ex[:, 0:kh, :], ee[:, 0:kh, :], Act.Exp)
                    # scale gathered h rows by exp (per head), in place.
                    # interleaved (c, h) layout keeps the broadcast operand's
                    # last dim packed -> DVE 2x mode
                    gh = G[:, 0:kh, 0:fo].rearrange("p k (c h) -> p k c h", h=H)
                    nc.vector.tensor_mul(
                        gh, gh,
                        ex[:, 0:kh, :].unsqueeze(2).broadcast_to([P, kh, C, H]))
                    # segment-sum via mask matmuls
                    for k in range(kh):
                        kk = k0 + k
                        st, sp = kk == 0, kk == kmax - 1
                        for si, (h0, hw_) in enumerate(segs):
                            nc.tensor.matmul(
                                pagg[si][:, 0:hw_], Sm[:, k, :],
                                G[:, k, h0:h0 + hw_],
                                start=st, stop=sp)
                        nc.tensor.matmul(pden[:], Sm[:, k, :], ex[:, k, :],
                                         start=st, stop=sp)

                # epilogue for block b
                rden = smallpool.tile([P, H], f32, tag="rden")
                nc.vector.reciprocal(rden[:], pden[:])
                yf = ypool.tile([P, fo], f32, tag="yf")
                for si, (h0, hw_) in enumerate(segs):
                    nch = hw_ // H
                    nc.vector.tensor_mul(
                        yf[:, h0:h0 + hw_].rearrange("p (c h) -> p c h", h=H),
                        pagg[si][:, 0:hw_].rearrange("p (c h) -> p c h", h=H),
                        rden[:].unsqueeze(1).broadcast_to([P, nch, H]))
                if not last:
                    nc.vector.tensor_add(yf[:], yf[:], skip_sb[:, b, :])
                    # ELU: y = max(yf,0) + exp(min(yf,0)) - 1
                    mn = ypool.tile([P, fo], f32, tag="mn")
                    nc.vector.tensor_scalar_min(mn[:], yf[:], 0.0)
                    nc.scalar.activation(mn[:], mn[:], Act.Exp)
                    nc.vector.scalar_tensor_tensor(
                        yf[:], yf[:], 0.0, mn[:], op0=Alu.max, op1=Alu.add)
                    nc.vector.tensor_scalar_add(yf[:], yf[:], -1.0)
                    # transpose into yT_new
                    for j in range(fo // P):
                        pt = psum_sm.tile([P, P], f32, tag="sm")
                        nc.tensor.transpose(pt[:], yf[:, j * P:(j + 1) * P],
                                            ident[:])
                        nc.vector.tensor_copy(yT_new[:, j, b * P:(b + 1) * P],
                                              pt[:])
                else:
                    # mean over heads + skip (interleaved: heads contiguous)
                    yo = ypool.tile([P, OUT_DIM], f32, tag="yo")
                    nc.vector.tensor_reduce(
                        yo[:], yf[:].rearrange("p (c h) -> p c h", h=H),
                        axis=mybir.AxisListType.X, op=Alu.add)
                    nc.vector.tensor_scalar_mul(yo[:], yo[:], 1.0 / H)
                    nc.vector.tensor_add(yo[:], yo[:], skip_sb[:, b, :])
                    nc.sync.dma_start(out_dram[b * P:(b + 1) * P, :], yo[:])

            yT_prev = yT_new

        if stage < 99 and stage != 5 or (stage <= 6 and stage >= 5):
            dummy = const_pool.tile([P, OUT_DIM], f32, tag="dummy")
            nc.vector.memset(dummy[:], 1.0)
            for b in range(npc // P):
                nc.sync.dma_start(out_dram[b * P:(b + 1) * P, :], dummy[:])

    nc.compile()
    return nc


# ---------------------------------------------------------------------------
# host wrapper
# ---------------------------------------------------------------------------

@functools.lru_cache(maxsize=2)
def _cached_program(n_pad, kmax):
    return build_program(n_pad, kmax)


def _replicate_row(v):
    v = np.asarray(v, np.float32).reshape(-1)
    return np.ascontiguousarray(np.broadcast_to(v[None, :], (P, v.shape[0])))


def make_in_maps(x, edge_index, weights):
    """weights: list of 3 dicts with keys W, linW, brow, aS, aD (numpy f32)."""
    n = x.shape[0]
    n_pad = cdiv(n, NCORES * P) * NCORES * P
    npc = n_pad // NCORES
    nb = npc // P

    g = _prep_graph(edge_index, n_pad)

    x_pad = np.zeros((n_pad, x.shape[1]), np.float32)
    x_pad[:n] = np.asarray(x, np.float32)
    xT_all = np.ascontiguousarray(x_pad.T)

    layer_w = []
    perm_prev = None  # input-feature (row) permutation from previous layer
    for li, lw in enumerate(weights):
        wdt = BF16
        _, fo, H, C = LAYERS[li]
        perm = _ilperm(H, C)  # (c, h)-interleaved output columns
        W = np.asarray(lw["W"], np.float32)[:, perm]
        linW = np.asarray(lw["linW"], np.float32)
        brow = np.asarray(lw["brow"], np.float32)
        if li < 2:  # skip path shares the interleaved layout (f_skip == fo)
            linW = linW[:, perm]
            brow = brow[perm]
        if perm_prev is not None:  # previous layer's y arrives interleaved
            W = W[perm_prev, :]
            linW = linW[perm_prev, :]
        perm_prev = perm
        layer_w.append(dict(
            W=np.ascontiguousarray(W.astype(wdt)),
            linW=np.ascontiguousarray(linW.astype(wdt)),
            brow=np.ascontiguousarray(brow.astype(wdt)[None, :]),
            aS=np.asarray(lw["aS"], np.float32).reshape(-1)[perm].reshape(1, -1),
            aD=np.asarray(lw["aD"], np.float32).reshape(-1)[perm].reshape(1, -1),
        ))

    in_maps = []
    for c in range(NCORES):
        blo, bhi = c * nb, (c + 1) * nb
        nbc = bhi - blo
        kmax = g["kmax"]
        m = dict(
            xT=np.ascontiguousarray(
                xT_all[:, c * npc:(c + 1) * npc].astype(BF16)),
            idx_src=_wrap_idx(g["src_pad"][blo:bhi].reshape(-1)),
            idx_dst=_wrap_idx(g["dst_pad"][blo:bhi].reshape(-1)),
            dstloc=np.ascontiguousarray(
                g["dl"][blo:bhi].transpose(1, 0, 2).reshape(P, nbc * kmax)
                .astype(np.float32)),
        )
        for li, lw in enumerate(layer_w):
            for key in ("W", "linW", "brow", "aS", "aD"):
                m[f"{key}{li}" if key != "W" else f"W{li}"] = lw[key]
        in_maps.append(m)
    return in_maps, g, n_pad


def _weights_from_kwargs(W1, a1_src, a1_dst, b1, lin1_W, lin1_b,
                         W2, a2_src, a2_dst, b2, lin2_W, lin2_b,
                         W3, a3_src, a3_dst, b3, lin3_W, lin3_b):
    return [
        dict(W=W1, linW=lin1_W, brow=np.asarray(b1) + np.asarray(lin1_b),
             aS=a1_src, aD=a1_dst),
        dict(W=W2, linW=lin2_W, brow=np.asarray(b2) + np.asarray(lin2_b),
             aS=a2_src, aD=a2_dst),
        dict(W=W3, linW=lin3_W, brow=np.asarray(b3) + np.asarray(lin3_b),
             aS=a3_src, aD=a3_dst),
    ]


def run_gat(inputs, trace=False, **run_kwargs):
    from concourse.bass_utils import run_bass_kernel_spmd

    kw = {k: inputs[k] for k in (
        "W1", "a1_src", "a1_dst", "b1", "lin1_W", "lin1_b",
        "W2", "a2_src", "a2_dst", "b2", "lin2_W", "lin2_b",
        "W3", "a3_src", "a3_dst", "b3", "lin3_W", "lin3_b")}
    weights = _weights_from_kwargs(**kw)
    x, edge_index = inputs["x"], inputs["edge_index"]
    in_maps, g, n_pad = make_in_maps(x, edge_index, weights)
    nc = _cached_program(n_pad, g["kmax"])
    res = run_bass_kernel_spmd(nc, in_maps, list(range(NCORES)),
                               trace=trace, **run_kwargs)
    out = np.concatenate([res.results[c]["out"] for c in range(NCORES)],
                         axis=0)
    n = x.shape[0]
    return np.ascontiguousarray(out[:n]).astype(np.float32), res


def kernel(**inputs):
    return run_gat(inputs)[0]



# revision 15
# speedup vs baseline: 1.1823x; 1.0343x over previous
"""3-layer GAT on Trainium2, 8 NeuronCores.

Strategy (dst-sharded):
  - Nodes padded to NPAD (mult of 8*128); core c owns a contiguous range of
    NPC nodes.  All edges (incl. self-loops on every padded node) are routed
    to the core that owns their *destination*, sorted by dst, grouped into
    dst-blocks of 128 destination nodes, and padded to chunks of 128 edges.
  - Per layer: each core computes h = y @ W for its own node rows (PE,
    bf16/f32r), plus per-head attention logits s,d (DVE).  It writes a
    "combined row" per node to DRAM: [h (bf16), s|d (f32 tail)], then an
    AllGather replicates the combined rows of all nodes to every core.
  - Edge phase: per dst-block, one dma_gather pulls the combined rows of the
    edge sources (h[src], s[src]) and a second tiny dma_gather pulls the
    tails of the edge destinations (d[dst]).  Softmax numerator
    exp(leaky_relu(s+d) - c) is computed per edge (c = per-core scalar
    upper bound on logits; softmax is shift-invariant so a per-core
    constant is exact since all edges of a dst live on one core).  The
    weighted segment-sum over incoming edges is a PE matmul with a
    host-precomputed one-hot mask S[e, dst_local], with exp folded into the
    gathered rows; the denominator uses the same mask with rhs = exp.
    Division, skip connection (y @ lin_W + b, PE), and ELU happen in the
    per-block epilogue; layer output is transposed (PE) into feat-major yT
    for the next layer's matmuls.
  - Layer 3: concat=False -> mean over 6 heads, no ELU; per-core rows DMA'd
    out, host concatenates and drops padding.
"""

import functools
import numpy as np
from contextlib import ExitStack

import ml_dtypes
import concourse.bass as bass
import concourse.bacc as bacc
import concourse.tile as tile
import concourse.masks as masks
from concourse import mybir
from concourse import library_config
from concourse._compat import cdiv

dt = mybir.dt
Alu = mybir.AluOpType
Act = mybir.ActivationFunctionType

BF16 = np.dtype(ml_dtypes.bfloat16)
NCORES = 8
P = 128

# layer configs: (F_in, F_out=H*C, H, C)
LAYERS = [
    (128, 1024, 4, 256),
    (1024, 1024, 4, 256),
    (1024, 384, 6, 64),
]
NEG_SLOPE = 0.2
OUT_DIM = 64


def _tail(H):
    # tail units (bf16) appended to h in each combined row; holds s|d logits
    # as bitcast f32. dma_gather requires gathered rows to be 256B multiples,
    # so the tail is 128 units (256 B).
    return 128


def _ilperm(H, C):
    # column permutation: interleaved col j = c*H + h  <-  head-major h*C + c
    j = np.arange(H * C)
    return (j % H) * C + j // H


# ---------------------------------------------------------------------------
# host-side graph preprocessing
# ---------------------------------------------------------------------------

def _prep_graph(edge_index, n_pad):
    """Sort edges (plus self-loops on all padded nodes) by dst; bucket into
    dst-blocks of 128; pad each block's edge list to a globally uniform
    multiple of 128 (KMAX chunks, SPMD uniformity across cores)."""
    src = np.asarray(edge_index[0], dtype=np.int64)
    dst = np.asarray(edge_index[1], dtype=np.int64)
    loops = np.arange(n_pad, dtype=np.int64)
    src = np.concatenate([src, loops])
    dst = np.concatenate([dst, loops])

    order = np.argsort(dst, kind="stable")
    src, dst = src[order], dst[order]

    nblocks = n_pad // P  # global dst blocks
    blk = dst // P
    counts = np.bincount(blk, minlength=nblocks)
    kmax = int(cdiv(int(counts.max()), P))
    neb = kmax * P  # edges per block (padded)

    src_pad = np.zeros((nblocks, neb), dtype=np.int64)
    dst_pad = np.zeros((nblocks, neb), dtype=np.int64)
    valid = np.zeros((nblocks, neb), dtype=bool)
    starts = np.concatenate([[0], np.cumsum(counts)])
    for b in range(nblocks):
        c = counts[b]
        s0 = starts[b]
        src_pad[b, :c] = src[s0:s0 + c]
        dst_pad[b, :c] = dst[s0:s0 + c]
        valid[b, :c] = True

    # one-hot masks, layout [nblocks, 128 (e within chunk), kmax, 128 (m)]
    dst_local = (dst_pad - (np.arange(nblocks) * P)[:, None]).astype(np.int64)
    dst_local[~valid] = -1
    # per-edge local dst index in e-partition-major layout [nblocks, 128, kmax]
    dl = dst_local.reshape(nblocks, kmax, P).transpose(0, 2, 1)
    dl = np.ascontiguousarray(dl.astype(np.int16))

    return dict(kmax=kmax, neb=neb, src_pad=src_pad, dst_pad=dst_pad, dl=dl)


def _wrap_idx(a):
    # [n] int -> [16, n//16] int16 (wrapped in 16 partitions; device replicates)
    n = a.shape[0]
    assert n % 16 == 0
    w = a.reshape(n // 16, 16).T.astype(np.int16)
    return np.ascontiguousarray(w)


# ---------------------------------------------------------------------------
# bass program builder
# ---------------------------------------------------------------------------

def build_program(n_pad, kmax, stage=99):
    # stage gates for HW bisection: 1=phase1(L1) 2=+c 3=+AG 4=+skip
    # 5=+edge(L1) 6=+L2 99=full
    npc = n_pad // NCORES      # nodes per core
    nb = npc // P              # dst blocks per core
    ne = nb * kmax * P         # padded edges per core
    kq = cdiv(kmax, 4)         # gather piece size (chunks)
    pieces = [(k0, min(kq, kmax - k0)) for k0 in range(0, kmax, kq)]

    nc = bacc.Bacc("TRN2", target_bir_lowering=False, debug=False)

    f32, bf16, i16 = dt.float32, dt.bfloat16, dt.int16
    f32r = dt.float32r

    # ---------------- DRAM I/O ----------------
    xT = nc.dram_tensor("xT", [P, npc], bf16, kind="ExternalInput")
    W = []
    for li, (fi, fo, H, C) in enumerate(LAYERS):
        wdt = bf16
        f_skip = OUT_DIM if li == 2 else fo
        W.append(dict(
            W=nc.dram_tensor(f"W{li}", [fi, fo], wdt, kind="ExternalInput"),
            linW=nc.dram_tensor(f"linW{li}", [fi, f_skip], wdt,
                                kind="ExternalInput"),
            brow=nc.dram_tensor(f"brow{li}", [1, f_skip], wdt,
                                kind="ExternalInput"),
            aS=nc.dram_tensor(f"aS{li}", [1, fo], f32, kind="ExternalInput"),
            aD=nc.dram_tensor(f"aD{li}", [1, fo], f32, kind="ExternalInput"),
        ))
    idx_src = nc.dram_tensor("idx_src", [16, ne // 16], i16, kind="ExternalInput")
    idx_dst = nc.dram_tensor("idx_dst", [16, ne // 16], i16, kind="ExternalInput")
    dstloc = nc.dram_tensor("dstloc", [P, nb * kmax], f32, kind="ExternalInput")
    out_dram = nc.dram_tensor("out", [npc, OUT_DIM], f32, kind="ExternalOutput")

    comb_own, comb_full = [], []
    for li, (fi, fo, H, C) in enumerate(LAYERS):
        cw = fo + _tail(H)
        comb_own.append(nc.dram_tensor(f"comb_own{li}", [npc, cw], bf16))
        comb_full.append(
            nc.dram_tensor(f"comb_full{li}", [n_pad, cw], bf16,
                           addr_space="Shared"))

    replica_groups = [list(range(NCORES))]

    with tile.TileContext(nc) as tc, ExitStack() as ctx:
        const_pool = ctx.enter_context(tc.tile_pool(name="const", bufs=1))
        wpool = ctx.enter_context(tc.tile_pool(name="w", bufs=1))
        hpool = ctx.enter_context(tc.tile_pool(name="h", bufs=2))
        sdpool = ctx.enter_context(tc.tile_pool(name="sd", bufs=3))
        scrpool = ctx.enter_context(tc.tile_pool(name="scr", bufs=1))
        gpool = ctx.enter_context(tc.tile_pool(name="g", bufs=4))
        tpool = ctx.enter_context(tc.tile_pool(name="t", bufs=4))
        mpool = ctx.enter_context(tc.tile_pool(name="m", bufs=4))
        epool = ctx.enter_context(tc.tile_pool(name="e", bufs=4))
        ypool = ctx.enter_context(tc.tile_pool(name="y", bufs=2))
        yTpool = ctx.enter_context(tc.tile_pool(name="yT", bufs=1))
        skippool = ctx.enter_context(tc.tile_pool(name="skip", bufs=1))
        smallpool = ctx.enter_context(tc.tile_pool(name="small", bufs=4))
        psum_mm = ctx.enter_context(tc.tile_pool(name="psmm", bufs=2, space="PSUM"))
        psum_agg = ctx.enter_context(tc.tile_pool(name="psagg", bufs=1, space="PSUM"))
        psum_den = ctx.enter_context(tc.tile_pool(name="psden", bufs=1, space="PSUM"))
        psum_sm = ctx.enter_context(tc.tile_pool(name="pssm", bufs=1, space="PSUM"))

        nc.gpsimd.load_library(library_config.mlp)

        # constants
        ident = const_pool.tile([P, P], f32)
        masks.make_identity(nc, ident[:])
        ones_f32 = const_pool.tile([1, P], f32)
        nc.vector.memset(ones_f32[:], 1.0)
        ones_bf = const_pool.tile([1, P], bf16)
        nc.vector.memset(ones_bf[:], 1.0)

        # index tiles (persistent); replicate [16, C] -> [128, C] on device
        idxs_t = const_pool.tile([P, ne // 16], i16, tag="idxs")
        idxd_t = const_pool.tile([P, ne // 16], i16, tag="idxd")
        # load 16 partitions once, then double on-chip (16->32->64->128)
        nc.sync.dma_start(idxs_t[0:16, :], idx_src[:])
        nc.sync.dma_start(idxd_t[0:16, :], idx_dst[:])
        for gi in (16, 32, 64):
            nc.sync.dma_start(idxs_t[gi:2 * gi, :], idxs_t[0:gi, :])
            nc.sync.dma_start(idxd_t[gi:2 * gi, :], idxd_t[0:gi, :])
        dstloc_t = const_pool.tile([P, nb * kmax], f32, tag="dstloc")
        nc.sync.dma_start(dstloc_t[:], dstloc[:])
        iot32 = const_pool.tile([P, P], dt.int32, tag="iot32")
        nc.gpsimd.iota(iot32[:], pattern=[[1, P]], base=0, channel_multiplier=0)
        iot = const_pool.tile([P, P], bf16, tag="iot")
        nc.vector.tensor_copy(iot[:], iot32[:])

        # xT resident for layer 1 (lhsT, f32)
        xT_sb = const_pool.tile([P, npc], bf16, tag="xT")
        nc.sync.dma_start(xT_sb[:], xT[:])

        yT_prev = None  # bf16 [128, fi//128, npc] for layers 2/3

        nlayers = (0 if stage == 0 else 1) if stage <= 5 else (2 if stage == 6 else len(LAYERS))
        for li, (fi, fo, H, C) in enumerate(LAYERS[:nlayers]):
            wdt = bf16
            kin = fi // P
            TAIL = _tail(H)
            cw = fo + TAIL
            tail0 = fo
            f_skip = OUT_DIM if li == 2 else fo
            last = li == 2
            ones_row = ones_f32 if wdt == f32 else ones_bf

            def mm(ap):
                return ap

            def lhs(k, b):
                if li == 0:
                    return xT_sb[:, b * P:(b + 1) * P]
                return yT_prev[:, k, b * P:(b + 1) * P]

            # -------- load weights for this layer --------
            W_sb = wpool.tile([P, kin, fo], wdt, tag="W")
            nc.sync.dma_start(W_sb[:], W[li]["W"].rearrange("(k p) f -> p k f", p=P))
            linW_sb = wpool.tile([P, kin, f_skip], wdt, tag="linW")
            nc.sync.dma_start(
                linW_sb[:], W[li]["linW"].rearrange("(k p) f -> p k f", p=P))
            brow_sb = wpool.tile([1, f_skip], wdt, tag="brow")
            nc.sync.dma_start(brow_sb[:], W[li]["brow"][:])
            aS_sb = wpool.tile([P, fo], f32, tag="aS")
            aD_sb = wpool.tile([P, fo], f32, tag="aD")
            arow = wpool.tile([1, 2 * fo], f32, tag="arow")
            nc.sync.dma_start(arow[:, 0:fo], W[li]["aS"][:])
            nc.sync.dma_start(arow[:, fo:2 * fo], W[li]["aD"][:])
            for dst_sb, off in ((aS_sb, 0), (aD_sb, fo)):
                for h0 in range(0, fo, 512):
                    hw_ = min(512, fo - h0)
                    pa = psum_mm.tile([P, 512], f32, tag="mm", name="pa")
                    nc.tensor.matmul(pa[:, 0:hw_], ones_f32[:],
                                     arow[:, off + h0:off + h0 + hw_],
                                     start=True, stop=True)
                    nc.vector.tensor_copy(dst_sb[:, h0:h0 + hw_], pa[:, 0:hw_])

            # running per-partition max of s and d
            smax = smallpool.tile([P, 2], f32, tag="smax")

            # -------- phase 1: h = y@W, s/d logits, comb rows --------
            segs = [(h0, min(512, fo - h0)) for h0 in range(0, fo, 512)]
            for b in range(nb):
                ph = [psum_mm.tile([P, 512], f32, tag="mm", name=f"ph{si}")
                      for si in range(len(segs))]
                for si, (h0, hw_) in enumerate(segs):
                    for k in range(kin):
                        nc.tensor.matmul(
                            ph[si][:, 0:hw_], mm(lhs(k, b)),
                            mm(W_sb[:, k, h0:h0 + hw_]),
                            start=(k == 0), stop=(k == kin - 1))
                # s/d logits: h * a, then per-head reduce over the strided
                # (c-major interleaved) view
                sd = sdpool.tile([P, 2 * H], f32, tag="sd")
                scr = scrpool.tile([P, 2, fo], f32, tag="sdscr")
                for j, aB in enumerate((aS_sb, aD_sb)):
                    for si, (h0, hw_) in enumerate(segs):
                        nc.vector.tensor_mul(scr[:, j, h0:h0 + hw_],
                                             ph[si][:, 0:hw_],
                                             aB[:, h0:h0 + hw_])
                    nc.vector.tensor_reduce(
                        sd[:, j * H:(j + 1) * H].unsqueeze(2),
                        scr[:, j, :].rearrange("p (c h) -> p h c", h=H),
                        axis=mybir.AxisListType.X, op=Alu.add)
                # track running max over (s cols, d cols)
                red = smallpool.tile([P, 2], f32, tag="red")
                nc.vector.tensor_reduce(red[:, 0:1], sd[:, 0:H],
                                        axis=mybir.AxisListType.X, op=Alu.max)
                nc.vector.tensor_reduce(red[:, 1:2], sd[:, H:2 * H],
                                        axis=mybir.AxisListType.X, op=Alu.max)
                if b == 0:
                    nc.vector.tensor_copy(smax[:], red[:])
                else:
                    nc.vector.tensor_max(smax[:], smax[:], red[:])
                # h -> bf16, write comb rows
                hbf = hpool.tile([P, fo], bf16, tag="hbf")
                for si, (h0, hw_) in enumerate(segs):
                    nc.scalar.activation(hbf[:, h0:h0 + hw_], ph[si][:, 0:hw_],
                                         Act.Copy)
                nc.sync.dma_start(comb_own[li][b * P:(b + 1) * P, 0:fo], hbf[:])
                nc.sync.dma_start(
                    comb_own[li][b * P:(b + 1) * P, tail0:tail0 + 4 * H],
                    sd[:].bitcast(bf16))

            if stage < 2 and li == nlayers - 1:
                break
            # -------- phase 2: scalar logit bound c --------
            csum = smallpool.tile([P, 1], f32, tag="csum")
            nc.vector.tensor_add(csum[:], smax[:, 0:1], smax[:, 1:2])
            ct = psum_sm.tile([1, P], f32, tag="sm")
            nc.tensor.transpose(ct[:], csum[:], ident[:])
            c1 = smallpool.tile([1, 1], f32, tag="c1")
            nc.vector.tensor_reduce(c1[:], ct[:], axis=mybir.AxisListType.X,
                                    op=Alu.max)
            pc = psum_sm.tile([P, 1], f32, tag="sm")
            nc.tensor.matmul(pc[:], ones_f32[:], c1[:], start=True, stop=True)
            cP = smallpool.tile([P, 1], f32, tag="cP")
            nc.vector.tensor_copy(cP[:], pc[:])

            if stage < 3 and li == nlayers - 1:
                break
            # -------- phase 3: AllGather combined rows --------
            nc.gpsimd.collective_compute(
                "AllGather", Alu.bypass, replica_groups=replica_groups,
                ins=[comb_own[li][:]], outs=[comb_full[li][:]])

            if stage < 4 and li == nlayers - 1:
                break
            # -------- phase 4: skip GEMM (overlaps AllGather) --------
            skip_sb = skippool.tile([P, nb, f_skip], bf16, tag="skip")
            sksegs = [(h0, min(512, f_skip - h0)) for h0 in range(0, f_skip, 512)]
            for b in range(nb):
                ps = [psum_mm.tile([P, 512], f32, tag="mm", name=f"ps{si}")
                      for si in range(len(sksegs))]
                for si, (h0, hw_) in enumerate(sksegs):
                    for k in range(kin):
                        nc.tensor.matmul(
                            ps[si][:, 0:hw_], mm(lhs(k, b)),
                            mm(linW_sb[:, k, h0:h0 + hw_]),
                            start=(k == 0), stop=False)
                    nc.tensor.matmul(
                        ps[si][:, 0:hw_], mm(ones_row[:]),
                        mm(brow_sb[:, h0:h0 + hw_]),
                        start=False, stop=True)
                    nc.scalar.activation(skip_sb[:, b, h0:h0 + hw_],
                                         ps[si][:, 0:hw_], Act.Copy)

            if stage < 5 and li == nlayers - 1:
                break
            # -------- phase 5: edge phase per dst block --------
            if not last:
                yT_new = yTpool.tile([P, fo // P, npc], bf16,
                                     tag=f"yT{li % 2}", name=f"yT_new{li}")
            else:
                yT_new = None
            comb_ap = comb_full[li]
            for b in range(nb):
                pagg = [psum_agg.tile([P, 512], f32, tag="pagg",
                                      name=f"pagg{si}")
                        for si in range(len(segs))]
                pden = psum_den.tile([P, H], f32, tag="pden")
                for (k0, kh) in pieces:
                    e0 = (b * kmax + k0) * P
                    n_idx = kh * P
                    G = gpool.tile([P, kq, cw], bf16, tag="G")
                    nc.gpsimd.dma_gather(
                        G[:, 0:kh, :], comb_ap[:, :],
                        idxs_t[:, e0 // 16:(e0 + n_idx) // 16],
                        n_idx, n_idx, cw, single_packet=False)
                    T = tpool.tile([P, kq, TAIL], bf16, tag="T")
                    nc.gpsimd.dma_gather(
                        T[:, 0:kh, :], comb_ap[:, tail0:tail0 + TAIL],
                        idxd_t[:, e0 // 16:(e0 + n_idx) // 16],
                        n_idx, n_idx, TAIL, elem_step=cw,
                        single_packet=False)
                    Sm = mpool.tile([P, kq, P], bf16, tag="Sm")
                    for k in range(kh):
                        gk = b * kmax + k0 + k
                        nc.vector.tensor_scalar(
                            Sm[:, k, :], iot[:], dstloc_t[:, gk:gk + 1], None,
                            op0=Alu.is_equal)

                    # logits -> exp
                    sv = G[:, 0:kh, fo:fo + 2 * H].bitcast(f32)   # [P, kh, H]
                    dv = T[:, 0:kh, 2 * H:4 * H].bitcast(f32)     # [P, kh, H]
                    ee = epool.tile([P, kq, H], f32, tag="ee")
                    nc.vector.tensor_add(ee[:, 0:kh, :], sv, dv)
                    nc.vector.scalar_tensor_tensor(
                        ee[:, 0:kh, :], ee[:, 0:kh, :], NEG_SLOPE,
                        ee[:, 0:kh, :], op0=Alu.mult, op1=Alu.max)
                    nc.vector.tensor_scalar(
                        ee[:, 0:kh, :], ee[:, 0:kh, :], cP[:, 0:1], 60.0,
                        op0=Alu.subtract, op1=Alu.min)
                    ex = epool.tile([P, kq, H], bf16, tag="ex")
                    nc.scalar.activation(ex[:, 0:kh, :], ee[:, 0:kh, :], Act.Exp)
                    # scale gathered h rows by exp (per head), in place.
                    # interleaved (c, h) layout keeps the broadcast operand's
                    # last dim packed -> DVE 2x mode
                    gh = G[:, 0:kh, 0:fo].rearrange("p k (c h) -> p k c h", h=H)
                    nc.vector.tensor_mul(
                        gh, gh,
                        ex[:, 0:kh, :].unsqueeze(2).broadcast_to([P, kh, C, H]))
                    # segment-sum via mask matmuls
                    for k in range(kh):
                        kk = k0 + k
                        st, sp = kk == 0, kk == kmax - 1
                        for si, (h0, hw_) in enumerate(segs):
                            nc.tensor.matmul(
                                pagg[si][:, 0:hw_], Sm[:, k, :],
                                G[:, k, h0:h0 + hw_],
                                start=st, stop=sp)
                        nc.tensor.matmul(pden[:], Sm[:, k, :], ex[:, k, :],
                                         start=st, stop=sp)

                # epilogue for block b
                rden = smallpool.tile([P, H], f32, tag="rden")
                nc.vector.reciprocal(rden[:], pden[:])
                yf = ypool.tile([P, fo], f32, tag="yf")
                for si, (h0, hw_) in enumerate(segs):
                    nch = hw_ // H
                    nc.vector.tensor_mul(
                        yf[:, h0:h0 + hw_].rearrange("p (c h) -> p c h", h=H),
                        pagg[si][:, 0:hw_].rearrange("p (c h) -> p c h", h=H),
                        rden[:].unsqueeze(1).broadcast_to([P, nch, H]))
                if not last:
                    nc.vector.tensor_add(yf[:], yf[:], skip_sb[:, b, :])
                    # ELU: y = max(yf,0) + exp(min(yf,0)) - 1
                    mn = ypool.tile([P, fo], f32, tag="mn")
                    nc.vector.tensor_scalar_min(mn[:], yf[:], 0.0)
                    nc.scalar.activation(mn[:], mn[:], Act.Exp)
                    nc.vector.scalar_tensor_tensor(
                        yf[:], yf[:], 0.0, mn[:], op0=Alu.max, op1=Alu.add)
                    nc.vector.tensor_scalar_add(yf[:], yf[:], -1.0)
                    # transpose into yT_new
                    for j in range(fo // P):
                        pt = psum_sm.tile([P, P], f32, tag="sm")
                        nc.tensor.transpose(pt[:], yf[:, j * P:(j + 1) * P],
                                            ident[:])
                        nc.scalar.activation(yT_new[:, j, b * P:(b + 1) * P],
                                             pt[:], Act.Copy)
                else:
                    # mean over heads + skip (interleaved: heads contiguous)
                    yo = ypool.tile([P, OUT_DIM], f32, tag="yo")
                    nc.vector.tensor_reduce(
                        yo[:], yf[:].rearrange("p (c h) -> p c h", h=H),
                        axis=mybir.AxisListType.X, op=Alu.add)
                    nc.vector.tensor_scalar_mul(yo[:], yo[:], 1.0 / H)
                    nc.vector.tensor_add(yo[:], yo[:], skip_sb[:, b, :])
                    nc.sync.dma_start(out_dram[b * P:(b + 1) * P, :], yo[:])

            yT_prev = yT_new

        if stage < 99 and stage != 5 or (stage <= 6 and stage >= 5):
            dummy = const_pool.tile([P, OUT_DIM], f32, tag="dummy")
            nc.vector.memset(dummy[:], 1.0)
            for b in range(npc // P):
                nc.sync.dma_start(out_dram[b * P:(b + 1) * P, :], dummy[:])

    nc.compile()
    return nc


# ---------------------------------------------------------------------------
# host wrapper
# ---------------------------------------------------------------------------

@functools.lru_cache(maxsize=2)
def _cached_program(n_pad, kmax):
    return build_program(n_pad, kmax)


def _replicate_row(v):
    v = np.asarray(v, np.float32).reshape(-1)
    return np.ascontiguousarray(np.broadcast_to(v[None, :], (P, v.shape[0])))


def make_in_maps(x, edge_index, weights):
    """weights: list of 3 dicts with keys W, linW, brow, aS, aD (numpy f32)."""
    n = x.shape[0]
    n_pad = cdiv(n, NCORES * P) * NCORES * P
    npc = n_pad // NCORES
    nb = npc // P

    g = _prep_graph(edge_index, n_pad)

    x_pad = np.zeros((n_pad, x.shape[1]), np.float32)
    x_pad[:n] = np.asarray(x, np.float32)
    xT_all = np.ascontiguousarray(x_pad.T)

    layer_w = []
    perm_prev = None  # input-feature (row) permutation from previous layer
    for li, lw in enumerate(weights):
        wdt = BF16
        _, fo, H, C = LAYERS[li]
        perm = _ilperm(H, C)  # (c, h)-interleaved output columns
        W = np.asarray(lw["W"], np.float32)[:, perm]
        linW = np.asarray(lw["linW"], np.float32)
        brow = np.asarray(lw["brow"], np.float32)
        if li < 2:  # skip path shares the interleaved layout (f_skip == fo)
            linW = linW[:, perm]
            brow = brow[perm]
        if perm_prev is not None:  # previous layer's y arrives interleaved
            W = W[perm_prev, :]
            linW = linW[perm_prev, :]
        perm_prev = perm
        layer_w.append(dict(
            W=np.ascontiguousarray(W.astype(wdt)),
            linW=np.ascontiguousarray(linW.astype(wdt)),
            brow=np.ascontiguousarray(brow.astype(wdt)[None, :]),
            aS=np.asarray(lw["aS"], np.float32).reshape(-1)[perm].reshape(1, -1),
            aD=np.asarray(lw["aD"], np.float32).reshape(-1)[perm].reshape(1, -1),
        ))

    in_maps = []
    for c in range(NCORES):
        blo, bhi = c * nb, (c + 1) * nb
        nbc = bhi - blo
        kmax = g["kmax"]
        m = dict(
            xT=np.ascontiguousarray(
                xT_all[:, c * npc:(c + 1) * npc].astype(BF16)),
            idx_src=_wrap_idx(g["src_pad"][blo:bhi].reshape(-1)),
            idx_dst=_wrap_idx(g["dst_pad"][blo:bhi].reshape(-1)),
            dstloc=np.ascontiguousarray(
                g["dl"][blo:bhi].transpose(1, 0, 2).reshape(P, nbc * kmax)
                .astype(np.float32)),
        )
        for li, lw in enumerate(layer_w):
            for key in ("W", "linW", "brow", "aS", "aD"):
                m[f"{key}{li}" if key != "W" else f"W{li}"] = lw[key]
        in_maps.append(m)
    return in_maps, g, n_pad


def _weights_from_kwargs(W1, a1_src, a1_dst, b1, lin1_W, lin1_b,
                         W2, a2_src, a2_dst, b2, lin2_W, lin2_b,
                         W3, a3_src, a3_dst, b3, lin3_W, lin3_b):
    return [
        dict(W=W1, linW=lin1_W, brow=np.asarray(b1) + np.asarray(lin1_b),
             aS=a1_src, aD=a1_dst),
        dict(W=W2, linW=lin2_W, brow=np.asarray(b2) + np.asarray(lin2_b),
             aS=a2_src, aD=a2_dst),
        dict(W=W3, linW=lin3_W, brow=np.asarray(b3) + np.asarray(lin3_b),
             aS=a3_src, aD=a3_dst),
    ]


def run_gat(inputs, trace=False, **run_kwargs):
    from concourse.bass_utils import run_bass_kernel_spmd

    kw = {k: inputs[k] for k in (
        "W1", "a1_src", "a1_dst", "b1", "lin1_W", "lin1_b",
        "W2", "a2_src", "a2_dst", "b2", "lin2_W", "lin2_b",
        "W3", "a3_src", "a3_dst", "b3", "lin3_W", "lin3_b")}
    weights = _weights_from_kwargs(**kw)
    x, edge_index = inputs["x"], inputs["edge_index"]
    in_maps, g, n_pad = make_in_maps(x, edge_index, weights)
    nc = _cached_program(n_pad, g["kmax"])
    res = run_bass_kernel_spmd(nc, in_maps, list(range(NCORES)),
                               trace=trace, **run_kwargs)
    out = np.concatenate([res.results[c]["out"] for c in range(NCORES)],
                         axis=0)
    n = x.shape[0]
    return np.ascontiguousarray(out[:n]).astype(np.float32), res


def kernel(**inputs):
    return run_gat(inputs)[0]



# revision 16
# speedup vs baseline: 1.1920x; 1.0082x over previous
"""3-layer GAT on Trainium2, 8 NeuronCores.

Strategy (dst-sharded):
  - Nodes padded to NPAD (mult of 8*128); core c owns a contiguous range of
    NPC nodes.  All edges (incl. self-loops on every padded node) are routed
    to the core that owns their *destination*, sorted by dst, grouped into
    dst-blocks of 128 destination nodes, and padded to chunks of 128 edges.
  - Per layer: each core computes h = y @ W for its own node rows (PE,
    bf16/f32r), plus per-head attention logits s,d (DVE).  It writes a
    "combined row" per node to DRAM: [h (bf16), s|d (f32 tail)], then an
    AllGather replicates the combined rows of all nodes to every core.
  - Edge phase: per dst-block, one dma_gather pulls the combined rows of the
    edge sources (h[src], s[src]) and a second tiny dma_gather pulls the
    tails of the edge destinations (d[dst]).  Softmax numerator
    exp(leaky_relu(s+d) - c) is computed per edge (c = per-core scalar
    upper bound on logits; softmax is shift-invariant so a per-core
    constant is exact since all edges of a dst live on one core).  The
    weighted segment-sum over incoming edges is a PE matmul with a
    host-precomputed one-hot mask S[e, dst_local], with exp folded into the
    gathered rows; the denominator uses the same mask with rhs = exp.
    Division, skip connection (y @ lin_W + b, PE), and ELU happen in the
    per-block epilogue; layer output is transposed (PE) into feat-major yT
    for the next layer's matmuls.
  - Layer 3: concat=False -> mean over 6 heads, no ELU; per-core rows DMA'd
    out, host concatenates and drops padding.
"""

import functools
import numpy as np
from contextlib import ExitStack

import ml_dtypes
import concourse.bass as bass
import concourse.bacc as bacc
import concourse.tile as tile
import concourse.masks as masks
from concourse import mybir
from concourse import library_config
from concourse._compat import cdiv

dt = mybir.dt
Alu = mybir.AluOpType
Act = mybir.ActivationFunctionType

BF16 = np.dtype(ml_dtypes.bfloat16)
NCORES = 8
P = 128

# layer configs: (F_in, F_out=H*C, H, C)
LAYERS = [
    (128, 1024, 4, 256),
    (1024, 1024, 4, 256),
    (1024, 384, 6, 64),
]
NEG_SLOPE = 0.2
OUT_DIM = 64


def _tail(H):
    # tail units (bf16) appended to h in each combined row; holds s|d logits
    # as bitcast f32. dma_gather requires gathered rows to be 256B multiples,
    # so the tail is 128 units (256 B).
    return 128


def _ilperm(H, C):
    # column permutation: interleaved col j = c*H + h  <-  head-major h*C + c
    j = np.arange(H * C)
    return (j % H) * C + j // H


# ---------------------------------------------------------------------------
# host-side graph preprocessing
# ---------------------------------------------------------------------------

def _prep_graph(edge_index, n_pad):
    """Sort edges (plus self-loops on all padded nodes) by dst; bucket into
    dst-blocks of 128; pad each block's edge list to a globally uniform
    multiple of 128 (KMAX chunks, SPMD uniformity across cores)."""
    src = np.asarray(edge_index[0], dtype=np.int64)
    dst = np.asarray(edge_index[1], dtype=np.int64)
    loops = np.arange(n_pad, dtype=np.int64)
    src = np.concatenate([src, loops])
    dst = np.concatenate([dst, loops])

    order = np.argsort(dst, kind="stable")
    src, dst = src[order], dst[order]

    nblocks = n_pad // P  # global dst blocks
    blk = dst // P
    counts = np.bincount(blk, minlength=nblocks)
    kmax = int(cdiv(int(counts.max()), P))
    neb = kmax * P  # edges per block (padded)

    src_pad = np.zeros((nblocks, neb), dtype=np.int64)
    dst_pad = np.zeros((nblocks, neb), dtype=np.int64)
    valid = np.zeros((nblocks, neb), dtype=bool)
    starts = np.concatenate([[0], np.cumsum(counts)])
    for b in range(nblocks):
        c = counts[b]
        s0 = starts[b]
        src_pad[b, :c] = src[s0:s0 + c]
        dst_pad[b, :c] = dst[s0:s0 + c]
        valid[b, :c] = True

    # one-hot masks, layout [nblocks, 128 (e within chunk), kmax, 128 (m)]
    dst_local = (dst_pad - (np.arange(nblocks) * P)[:, None]).astype(np.int64)
    dst_local[~valid] = -1
    # per-edge local dst index in e-partition-major layout [nblocks, 128, kmax]
    dl = dst_local.reshape(nblocks, kmax, P).transpose(0, 2, 1)
    dl = np.ascontiguousarray(dl.astype(np.int16))

    return dict(kmax=kmax, neb=neb, src_pad=src_pad, dst_pad=dst_pad, dl=dl)


def _wrap_idx(a):
    # [n] int -> [16, n//16] int16 (wrapped in 16 partitions; device replicates)
    n = a.shape[0]
    assert n % 16 == 0
    w = a.reshape(n // 16, 16).T.astype(np.int16)
    return np.ascontiguousarray(w)


# ---------------------------------------------------------------------------
# bass program builder
# ---------------------------------------------------------------------------

def build_program(n_pad, kmax, stage=99):
    # stage gates for HW bisection: 1=phase1(L1) 2=+c 3=+AG 4=+skip
    # 5=+edge(L1) 6=+L2 99=full
    npc = n_pad // NCORES      # nodes per core
    nb = npc // P              # dst blocks per core
    ne = nb * kmax * P         # padded edges per core
    kq = cdiv(kmax, 4)         # gather piece size (chunks)
    pieces = [(k0, min(kq, kmax - k0)) for k0 in range(0, kmax, kq)]

    nc = bacc.Bacc("TRN2", target_bir_lowering=False, debug=False)

    f32, bf16, i16 = dt.float32, dt.bfloat16, dt.int16
    f32r = dt.float32r

    # ---------------- DRAM I/O ----------------
    xT = nc.dram_tensor("xT", [P, npc], bf16, kind="ExternalInput")
    W = []
    for li, (fi, fo, H, C) in enumerate(LAYERS):
        wdt = bf16
        f_skip = OUT_DIM if li == 2 else fo
        W.append(dict(
            W=nc.dram_tensor(f"W{li}", [fi, fo], wdt, kind="ExternalInput"),
            linW=nc.dram_tensor(f"linW{li}", [fi, f_skip], wdt,
                                kind="ExternalInput"),
            brow=nc.dram_tensor(f"brow{li}", [1, f_skip], wdt,
                                kind="ExternalInput"),
            aS=nc.dram_tensor(f"aS{li}", [1, fo], f32, kind="ExternalInput"),
            aD=nc.dram_tensor(f"aD{li}", [1, fo], f32, kind="ExternalInput"),
        ))
    idx_src = nc.dram_tensor("idx_src", [16, ne // 16], i16, kind="ExternalInput")
    idx_dst = nc.dram_tensor("idx_dst", [16, ne // 16], i16, kind="ExternalInput")
    dstloc = nc.dram_tensor("dstloc", [P, nb * kmax], f32, kind="ExternalInput")
    out_dram = nc.dram_tensor("out", [npc, OUT_DIM], f32, kind="ExternalOutput")

    comb_own, comb_full = [], []
    for li, (fi, fo, H, C) in enumerate(LAYERS):
        cw = fo + _tail(H)
        comb_own.append(nc.dram_tensor(f"comb_own{li}", [npc, cw], bf16))
        comb_full.append(
            nc.dram_tensor(f"comb_full{li}", [n_pad, cw], bf16,
                           addr_space="Shared"))

    replica_groups = [list(range(NCORES))]

    with tile.TileContext(nc) as tc, ExitStack() as ctx:
        const_pool = ctx.enter_context(tc.tile_pool(name="const", bufs=1))
        wpool = ctx.enter_context(tc.tile_pool(name="w", bufs=1))
        hpool = ctx.enter_context(tc.tile_pool(name="h", bufs=2))
        sdpool = ctx.enter_context(tc.tile_pool(name="sd", bufs=3))
        scrpool = ctx.enter_context(tc.tile_pool(name="scr", bufs=1))
        gpool = ctx.enter_context(tc.tile_pool(name="g", bufs=4))
        tpool = ctx.enter_context(tc.tile_pool(name="t", bufs=4))
        mpool = ctx.enter_context(tc.tile_pool(name="m", bufs=4))
        epool = ctx.enter_context(tc.tile_pool(name="e", bufs=4))
        ypool = ctx.enter_context(tc.tile_pool(name="y", bufs=2))
        yTpool = ctx.enter_context(tc.tile_pool(name="yT", bufs=1))
        skippool = ctx.enter_context(tc.tile_pool(name="skip", bufs=1))
        smallpool = ctx.enter_context(tc.tile_pool(name="small", bufs=4))
        psum_mm = ctx.enter_context(tc.tile_pool(name="psmm", bufs=2, space="PSUM"))
        psum_agg = ctx.enter_context(tc.tile_pool(name="psagg", bufs=1, space="PSUM"))
        psum_den = ctx.enter_context(tc.tile_pool(name="psden", bufs=1, space="PSUM"))
        psum_sm = ctx.enter_context(tc.tile_pool(name="pssm", bufs=1, space="PSUM"))

        nc.gpsimd.load_library(library_config.mlp)

        # constants
        ident = const_pool.tile([P, P], f32)
        masks.make_identity(nc, ident[:])
        ones_f32 = const_pool.tile([1, P], f32)
        nc.vector.memset(ones_f32[:], 1.0)
        ones_bf = const_pool.tile([1, P], bf16)
        nc.vector.memset(ones_bf[:], 1.0)

        # index tiles (persistent); replicate [16, C] -> [128, C] on device
        idxs_t = const_pool.tile([P, ne // 16], i16, tag="idxs")
        idxd_t = const_pool.tile([P, ne // 16], i16, tag="idxd")
        # load 16 partitions once, then double on-chip (16->32->64->128)
        nc.sync.dma_start(idxs_t[0:16, :], idx_src[:])
        nc.sync.dma_start(idxd_t[0:16, :], idx_dst[:])
        for gi in (16, 32, 64):
            nc.sync.dma_start(idxs_t[gi:2 * gi, :], idxs_t[0:gi, :])
            nc.sync.dma_start(idxd_t[gi:2 * gi, :], idxd_t[0:gi, :])
        dstloc_t = const_pool.tile([P, nb * kmax], f32, tag="dstloc")
        nc.sync.dma_start(dstloc_t[:], dstloc[:])
        iot32 = const_pool.tile([P, P], dt.int32, tag="iot32")
        nc.gpsimd.iota(iot32[:], pattern=[[1, P]], base=0, channel_multiplier=0)
        iot = const_pool.tile([P, P], bf16, tag="iot")
        nc.vector.tensor_copy(iot[:], iot32[:])

        # xT resident for layer 1 (lhsT, f32)
        xT_sb = const_pool.tile([P, npc], bf16, tag="xT")
        nc.sync.dma_start(xT_sb[:], xT[:])

        yT_prev = None  # bf16 [128, fi//128, npc] for layers 2/3

        nlayers = (0 if stage == 0 else 1) if stage <= 5 else (2 if stage == 6 else len(LAYERS))
        for li, (fi, fo, H, C) in enumerate(LAYERS[:nlayers]):
            wdt = bf16
            kin = fi // P
            TAIL = _tail(H)
            cw = fo + TAIL
            tail0 = fo
            f_skip = OUT_DIM if li == 2 else fo
            last = li == 2
            ones_row = ones_f32 if wdt == f32 else ones_bf

            def mm(ap):
                return ap

            def lhs(k, b):
                if li == 0:
                    return xT_sb[:, b * P:(b + 1) * P]
                return yT_prev[:, k, b * P:(b + 1) * P]

            # -------- load weights for this layer --------
            W_sb = wpool.tile([P, kin, fo], wdt, tag="W")
            nc.sync.dma_start(W_sb[:], W[li]["W"].rearrange("(k p) f -> p k f", p=P))
            linW_sb = wpool.tile([P, kin, f_skip], wdt, tag="linW")
            nc.sync.dma_start(
                linW_sb[:], W[li]["linW"].rearrange("(k p) f -> p k f", p=P))
            brow_sb = wpool.tile([1, f_skip], wdt, tag="brow")
            nc.sync.dma_start(brow_sb[:], W[li]["brow"][:])
            aS_sb = wpool.tile([P, fo], bf16, tag="aS")
            aD_sb = wpool.tile([P, fo], bf16, tag="aD")
            arow = wpool.tile([1, 2 * fo], f32, tag="arow")
            nc.sync.dma_start(arow[:, 0:fo], W[li]["aS"][:])
            nc.sync.dma_start(arow[:, fo:2 * fo], W[li]["aD"][:])
            for dst_sb, off in ((aS_sb, 0), (aD_sb, fo)):
                for h0 in range(0, fo, 512):
                    hw_ = min(512, fo - h0)
                    pa = psum_mm.tile([P, 512], f32, tag="mm", name="pa")
                    nc.tensor.matmul(pa[:, 0:hw_], ones_f32[:],
                                     arow[:, off + h0:off + h0 + hw_],
                                     start=True, stop=True)
                    nc.vector.tensor_copy(dst_sb[:, h0:h0 + hw_], pa[:, 0:hw_])

            # running per-partition max of s and d
            smax = smallpool.tile([P, 2], f32, tag="smax")

            # -------- phase 1: h = y@W, s/d logits, comb rows --------
            segs = [(h0, min(512, fo - h0)) for h0 in range(0, fo, 512)]
            for b in range(nb):
                ph = [psum_mm.tile([P, 512], f32, tag="mm", name=f"ph{si}")
                      for si in range(len(segs))]
                for si, (h0, hw_) in enumerate(segs):
                    for k in range(kin):
                        nc.tensor.matmul(
                            ph[si][:, 0:hw_], mm(lhs(k, b)),
                            mm(W_sb[:, k, h0:h0 + hw_]),
                            start=(k == 0), stop=(k == kin - 1))
                # h -> bf16 rows first (Act engine), then s/d logits from
                # the bf16 copy (DVE 2x muls + per-head strided reduces)
                hbf = hpool.tile([P, fo], bf16, tag="hbf")
                for si, (h0, hw_) in enumerate(segs):
                    nc.scalar.activation(hbf[:, h0:h0 + hw_], ph[si][:, 0:hw_],
                                         Act.Copy)
                sd = sdpool.tile([P, 2 * H], f32, tag="sd")
                scr = scrpool.tile([P, 2, fo], bf16, tag="sdscr")
                for j, aB in enumerate((aS_sb, aD_sb)):
                    nc.vector.tensor_mul(scr[:, j, :], hbf[:], aB[:])
                    nc.vector.tensor_reduce(
                        sd[:, j * H:(j + 1) * H].unsqueeze(2),
                        scr[:, j, :].rearrange("p (c h) -> p h c", h=H),
                        axis=mybir.AxisListType.X, op=Alu.add)
                # track running max over (s cols, d cols)
                red = smallpool.tile([P, 2], f32, tag="red")
                nc.vector.tensor_reduce(red[:, 0:1], sd[:, 0:H],
                                        axis=mybir.AxisListType.X, op=Alu.max)
                nc.vector.tensor_reduce(red[:, 1:2], sd[:, H:2 * H],
                                        axis=mybir.AxisListType.X, op=Alu.max)
                if b == 0:
                    nc.vector.tensor_copy(smax[:], red[:])
                else:
                    nc.vector.tensor_max(smax[:], smax[:], red[:])
                # write comb rows
                nc.sync.dma_start(comb_own[li][b * P:(b + 1) * P, 0:fo], hbf[:])
                nc.sync.dma_start(
                    comb_own[li][b * P:(b + 1) * P, tail0:tail0 + 4 * H],
                    sd[:].bitcast(bf16))

            if stage < 2 and li == nlayers - 1:
                break
            # -------- phase 2: scalar logit bound c --------
            csum = smallpool.tile([P, 1], f32, tag="csum")
            nc.vector.tensor_add(csum[:], smax[:, 0:1], smax[:, 1:2])
            ct = psum_sm.tile([1, P], f32, tag="sm")
            nc.tensor.transpose(ct[:], csum[:], ident[:])
            c1 = smallpool.tile([1, 1], f32, tag="c1")
            nc.vector.tensor_reduce(c1[:], ct[:], axis=mybir.AxisListType.X,
                                    op=Alu.max)
            pc = psum_sm.tile([P, 1], f32, tag="sm")
            nc.tensor.matmul(pc[:], ones_f32[:], c1[:], start=True, stop=True)
            cP = smallpool.tile([P, 1], f32, tag="cP")
            nc.vector.tensor_copy(cP[:], pc[:])

            if stage < 3 and li == nlayers - 1:
                break
            # -------- phase 3: AllGather combined rows --------
            nc.gpsimd.collective_compute(
                "AllGather", Alu.bypass, replica_groups=replica_groups,
                ins=[comb_own[li][:]], outs=[comb_full[li][:]])

            if stage < 4 and li == nlayers - 1:
                break
            # -------- phase 4: skip GEMM (overlaps AllGather) --------
            skip_sb = skippool.tile([P, nb, f_skip], bf16, tag="skip")
            sksegs = [(h0, min(512, f_skip - h0)) for h0 in range(0, f_skip, 512)]
            for b in range(nb):
                ps = [psum_mm.tile([P, 512], f32, tag="mm", name=f"ps{si}")
                      for si in range(len(sksegs))]
                for si, (h0, hw_) in enumerate(sksegs):
                    for k in range(kin):
                        nc.tensor.matmul(
                            ps[si][:, 0:hw_], mm(lhs(k, b)),
                            mm(linW_sb[:, k, h0:h0 + hw_]),
                            start=(k == 0), stop=False)
                    nc.tensor.matmul(
                        ps[si][:, 0:hw_], mm(ones_row[:]),
                        mm(brow_sb[:, h0:h0 + hw_]),
                        start=False, stop=True)
                    nc.scalar.activation(skip_sb[:, b, h0:h0 + hw_],
                                         ps[si][:, 0:hw_], Act.Copy)

            if stage < 5 and li == nlayers - 1:
                break
            # -------- phase 5: edge phase per dst block --------
            if not last:
                yT_new = yTpool.tile([P, fo // P, npc], bf16,
                                     tag=f"yT{li % 2}", name=f"yT_new{li}")
            else:
                yT_new = None
            comb_ap = comb_full[li]
            for b in range(nb):
                pagg = [psum_agg.tile([P, 512], f32, tag="pagg",
                                      name=f"pagg{si}")
                        for si in range(len(segs))]
                pden = psum_den.tile([P, H], f32, tag="pden")
                for (k0, kh) in pieces:
                    e0 = (b * kmax + k0) * P
                    n_idx = kh * P
                    G = gpool.tile([P, kq, cw], bf16, tag="G")
                    nc.gpsimd.dma_gather(
                        G[:, 0:kh, :], comb_ap[:, :],
                        idxs_t[:, e0 // 16:(e0 + n_idx) // 16],
                        n_idx, n_idx, cw, single_packet=False)
                    T = tpool.tile([P, kq, TAIL], bf16, tag="T")
                    nc.gpsimd.dma_gather(
                        T[:, 0:kh, :], comb_ap[:, tail0:tail0 + TAIL],
                        idxd_t[:, e0 // 16:(e0 + n_idx) // 16],
                        n_idx, n_idx, TAIL, elem_step=cw,
                        single_packet=False)
                    Sm = mpool.tile([P, kq, P], bf16, tag="Sm")
                    for k in range(kh):
                        gk = b * kmax + k0 + k
                        nc.vector.tensor_scalar(
                            Sm[:, k, :], iot[:], dstloc_t[:, gk:gk + 1], None,
                            op0=Alu.is_equal)

                    # logits -> exp
                    sv = G[:, 0:kh, fo:fo + 2 * H].bitcast(f32)   # [P, kh, H]
                    dv = T[:, 0:kh, 2 * H:4 * H].bitcast(f32)     # [P, kh, H]
                    ee = epool.tile([P, kq, H], f32, tag="ee")
                    nc.vector.tensor_add(ee[:, 0:kh, :], sv, dv)
                    nc.vector.scalar_tensor_tensor(
                        ee[:, 0:kh, :], ee[:, 0:kh, :], NEG_SLOPE,
                        ee[:, 0:kh, :], op0=Alu.mult, op1=Alu.max)
                    nc.vector.tensor_scalar(
                        ee[:, 0:kh, :], ee[:, 0:kh, :], cP[:, 0:1], 60.0,
                        op0=Alu.subtract, op1=Alu.min)
                    ex = epool.tile([P, kq, H], bf16, tag="ex")
                    nc.scalar.activation(ex[:, 0:kh, :], ee[:, 0:kh, :], Act.Exp)
                    # scale gathered h rows by exp (per head), in place.
                    # interleaved (c, h) layout keeps the broadcast operand's
                    # last dim packed -> DVE 2x mode
                    gh = G[:, 0:kh, 0:fo].rearrange("p k (c h) -> p k c h", h=H)
                    nc.vector.tensor_mul(
                        gh, gh,
                        ex[:, 0:kh, :].unsqueeze(2).broadcast_to([P, kh, C, H]))
                    # segment-sum via mask matmuls
                    for k in range(kh):
                        kk = k0 + k
                        st, sp = kk == 0, kk == kmax - 1
                        for si, (h0, hw_) in enumerate(segs):
                            nc.tensor.matmul(
                                pagg[si][:, 0:hw_], Sm[:, k, :],
                                G[:, k, h0:h0 + hw_],
                                start=st, stop=sp)
                        nc.tensor.matmul(pden[:], Sm[:, k, :], ex[:, k, :],
                                         start=st, stop=sp)

                # epilogue for block b
                rden = smallpool.tile([P, H], f32, tag="rden")
                nc.vector.reciprocal(rden[:], pden[:])
                yf = ypool.tile([P, fo], f32, tag="yf")
                for si, (h0, hw_) in enumerate(segs):
                    nch = hw_ // H
                    nc.vector.tensor_mul(
                        yf[:, h0:h0 + hw_].rearrange("p (c h) -> p c h", h=H),
                        pagg[si][:, 0:hw_].rearrange("p (c h) -> p c h", h=H),
                        rden[:].unsqueeze(1).broadcast_to([P, nch, H]))
                if not last:
                    nc.vector.tensor_add(yf[:], yf[:], skip_sb[:, b, :])
                    # ELU: y = max(yf,0) + exp(min(yf,0)) - 1
                    mn = ypool.tile([P, fo], f32, tag="mn")
                    nc.vector.tensor_scalar_min(mn[:], yf[:], 0.0)
                    nc.scalar.activation(mn[:], mn[:], Act.Exp)
                    nc.vector.scalar_tensor_tensor(
                        yf[:], yf[:], 0.0, mn[:], op0=Alu.max, op1=Alu.add)
                    # transpose into yT_new
                    for j in range(fo // P):
                        pt = psum_sm.tile([P, P], f32, tag="sm")
                        nc.tensor.transpose(pt[:], yf[:, j * P:(j + 1) * P],
                                            ident[:])
                        nc.scalar.activation(yT_new[:, j, b * P:(b + 1) * P],
                                             pt[:], Act.Copy, bias=-1.0)
                else:
                    # mean over heads + skip (interleaved: heads contiguous)
                    yo = ypool.tile([P, OUT_DIM], f32, tag="yo")
                    nc.vector.tensor_reduce(
                        yo[:], yf[:].rearrange("p (c h) -> p c h", h=H),
                        axis=mybir.AxisListType.X, op=Alu.add)
                    nc.vector.tensor_scalar_mul(yo[:], yo[:], 1.0 / H)
                    nc.vector.tensor_add(yo[:], yo[:], skip_sb[:, b, :])
                    nc.sync.dma_start(out_dram[b * P:(b + 1) * P, :], yo[:])

            yT_prev = yT_new

        if stage < 99 and stage != 5 or (stage <= 6 and stage >= 5):
            dummy = const_pool.tile([P, OUT_DIM], f32, tag="dummy")
            nc.vector.memset(dummy[:], 1.0)
            for b in range(npc // P):
                nc.sync.dma_start(out_dram[b * P:(b + 1) * P, :], dummy[:])

    nc.compile()
    return nc


# ---------------------------------------------------------------------------
# host wrapper
# ---------------------------------------------------------------------------

@functools.lru_cache(maxsize=2)
def _cached_program(n_pad, kmax):
    return build_program(n_pad, kmax)


def _replicate_row(v):
    v = np.asarray(v, np.float32).reshape(-1)
    return np.ascontiguousarray(np.broadcast_to(v[None, :], (P, v.shape[0])))


def make_in_maps(x, edge_index, weights):
    """weights: list of 3 dicts with keys W, linW, brow, aS, aD (numpy f32)."""
    n = x.shape[0]
    n_pad = cdiv(n, NCORES * P) * NCORES * P
    npc = n_pad // NCORES
    nb = npc // P

    g = _prep_graph(edge_index, n_pad)

    x_pad = np.zeros((n_pad, x.shape[1]), np.float32)
    x_pad[:n] = np.asarray(x, np.float32)
    xT_all = np.ascontiguousarray(x_pad.T)

    layer_w = []
    perm_prev = None  # input-feature (row) permutation from previous layer
    for li, lw in enumerate(weights):
        wdt = BF16
        _, fo, H, C = LAYERS[li]
        perm = _ilperm(H, C)  # (c, h)-interleaved output columns
        W = np.asarray(lw["W"], np.float32)[:, perm]
        linW = np.asarray(lw["linW"], np.float32)
        brow = np.asarray(lw["brow"], np.float32)
        if li < 2:  # skip path shares the interleaved layout (f_skip == fo)
            linW = linW[:, perm]
            brow = brow[perm]
        if perm_prev is not None:  # previous layer's y arrives interleaved
            W = W[perm_prev, :]
            linW = linW[perm_prev, :]
        perm_prev = perm
        layer_w.append(dict(
            W=np.ascontiguousarray(W.astype(wdt)),
            linW=np.ascontiguousarray(linW.astype(wdt)),
            brow=np.ascontiguousarray(brow.astype(wdt)[None, :]),
            aS=np.asarray(lw["aS"], np.float32).reshape(-1)[perm].reshape(1, -1),
            aD=np.asarray(lw["aD"], np.float32).reshape(-1)[perm].reshape(1, -1),
        ))

    in_maps = []
    for c in range(NCORES):
        blo, bhi = c * nb, (c + 1) * nb
        nbc = bhi - blo
        kmax = g["kmax"]
        m = dict(
            xT=np.ascontiguousarray(
                xT_all[:, c * npc:(c + 1) * npc].astype(BF16)),
            idx_src=_wrap_idx(g["src_pad"][blo:bhi].reshape(-1)),
            idx_dst=_wrap_idx(g["dst_pad"][blo:bhi].reshape(-1)),
            dstloc=np.ascontiguousarray(
                g["dl"][blo:bhi].transpose(1, 0, 2).reshape(P, nbc * kmax)
                .astype(np.float32)),
        )
        for li, lw in enumerate(layer_w):
            for key in ("W", "linW", "brow", "aS", "aD"):
                m[f"{key}{li}" if key != "W" else f"W{li}"] = lw[key]
        in_maps.append(m)
    return in_maps, g, n_pad


def _weights_from_kwargs(W1, a1_src, a1_dst, b1, lin1_W, lin1_b,
                         W2, a2_src, a2_dst, b2, lin2_W, lin2_b,
                         W3, a3_src, a3_dst, b3, lin3_W, lin3_b):
    return [
        dict(W=W1, linW=lin1_W, brow=np.asarray(b1) + np.asarray(lin1_b),
             aS=a1_src, aD=a1_dst),
        dict(W=W2, linW=lin2_W, brow=np.asarray(b2) + np.asarray(lin2_b),
             aS=a2_src, aD=a2_dst),
        dict(W=W3, linW=lin3_W, brow=np.asarray(b3) + np.asarray(lin3_b),
             aS=a3_src, aD=a3_dst),
    ]


def run_gat(inputs, trace=False, **run_kwargs):
    from concourse.bass_utils import run_bass_kernel_spmd

    kw = {k: inputs[k] for k in (
        "W1", "a1_src", "a1_dst", "b1", "lin1_W", "lin1_b",
        "W2", "a2_src", "a2_dst", "b2", "lin2_W", "lin2_b",
        "W3", "a3_src", "a3_dst", "b3", "lin3_W", "lin3_b")}
    weights = _weights_from_kwargs(**kw)
    x, edge_index = inputs["x"], inputs["edge_index"]
    in_maps, g, n_pad = make_in_maps(x, edge_index, weights)
    nc = _cached_program(n_pad, g["kmax"])
    res = run_bass_kernel_spmd(nc, in_maps, list(range(NCORES)),
                               trace=trace, **run_kwargs)
    out = np.concatenate([res.results[c]["out"] for c in range(NCORES)],
                         axis=0)
    n = x.shape[0]
    return np.ascontiguousarray(out[:n]).astype(np.float32), res


def kernel(**inputs):
    return run_gat(inputs)[0]

